# revision 8
# baseline (speedup 1.0000x reference)
# Trainium2 Bass kernel for nn_RPNmodel (RPN conv head + proposal decode + NMS).
#
# Device (8 NeuronCores, SPMD, feature rows sharded 7 rows/core):
#   - effective-weight contraction W_eff[k,o] = sum_c w1[c,k]*w2[o,c] on the PE,
#     using fp16 fixed-point slices with exact integer accumulation in PSUM
#   - conv-as-GEMM over the 9 taps: scores + locs for the core's positions (PE)
#   - box decode (polynomial exp), validity, int32 sort keys (DVE)
# Host: shards/pads inputs, slices weights into fixed-point fp16 words (a
# lossless re-encoding), gathers per-core outputs, and applies the greedy NMS
# ordering on the device-computed keys/boxes to emit rois.
#
# Precision: objectness scores are produced as exact-integer hi words (grid
# 2^-22) plus fp32 residuals => |score - exact| ~1e-9, so the descending key
# order reproduces the fp32 reference ordering wherever it is determined.
import numpy as np

FEAT_W = 52
IMG_W = 210.0
SUB = 4
NA = 9
N_CORES = 8
R = 7                  # feature rows per core (8*7 = 56 >= 52, tail zero-padded)
NPOS = R * FEAT_W      # 364 positions per core
PRE_NMS = 12000
POST_NMS = 2000

f32 = np.float32
MAGIC = float(3 * 2 ** 22)

# device channel order (54 rows), all 32-alignment-friendly groups:
#  0:9 obj | 9:18 cls-even | 18:27 l0 | 27:36 l1 | 36:45 l2 | 45:54 l3
def _chan_perm():
    reg = [a * 4 + d for d in range(4) for a in range(NA)]
    cls_even = [36 + 2 * a for a in range(NA)]
    cls_odd = [36 + 2 * a + 1 for a in range(NA)]
    return np.array(cls_odd + cls_even + reg, np.int64)

CHAN_PERM = _chan_perm()

def _magic_round(x, scale=1.0):
    m = f32(MAGIC)
    t = (x.astype(f32) * f32(scale) + m).astype(f32)
    return (t - m).astype(f32)

def _slice3(w, g1, g2, g3):
    a = _magic_round(w, 2.0 ** g1)
    b = _magic_round(w * f32(2.0 ** g2) - a * f32(2.0 ** (g2 - g1)), 1.0)
    c = _magic_round(w * f32(2.0 ** g3) - a * f32(2.0 ** (g3 - g1)) - b * f32(2.0 ** (g3 - g2)), 1.0)
    return a, b, c

def _make_anchor_geom():
    vals = (np.arange(SUB, (FEAT_W + 1) * SUB, SUB) - SUB // 2).astype(np.float64)
    ratios = np.array([0.5, 1.0, 2.0])
    scales = np.array([4.0, 8.0, 16.0])
    h = (SUB * scales[None, :] * np.sqrt(ratios[:, None])).reshape(-1)
    w = (SUB * scales[None, :] * np.sqrt(1.0 / ratios[:, None])).reshape(-1)
    return vals, h.astype(f32), w.astype(f32)

def _exp_poly():
    t = np.linspace(-0.37, 0.37, 4001)
    return np.polyfit(t, np.exp(t), 6).astype(np.float64)

EXP_CO = _exp_poly()

_BUILD_CACHE = {}

def _build():
    if "nc" in _BUILD_CACHE:
        return _BUILD_CACHE["nc"]
    import concourse.bacc as bacc
    import concourse.mybir as mybir
    from concourse.tile import TileContext

    dt = mybir.dt
    op = mybir.AluOpType
    nc = bacc.Bacc("TRN2", target_bir_lowering=False, debug=False, num_devices=N_CORES)

    x_in = nc.declare_dram_parameter("x_in", [128, 2 * 9 * 54], dt.float32, isOutput=False)
    w1a_in = nc.declare_dram_parameter("w1a", [128, 2 * 2304], dt.float16, isOutput=False)
    w1b_in = nc.declare_dram_parameter("w1b", [128, 2 * 2304], dt.float16, isOutput=False)
    w1c_in = nc.declare_dram_parameter("w1c", [128, 2 * 2304], dt.float16, isOutput=False)
    w2abc_in = nc.declare_dram_parameter("w2abc", [128, 2 * 162], dt.float16, isOutput=False)
    w2f_in = nc.declare_dram_parameter("w2f", [128, 2 * 54], dt.float32, isOutput=False)
    b1_in = nc.declare_dram_parameter("b1", [128, 2], dt.float32, isOutput=False)
    b2_in = nc.declare_dram_parameter("b2", [54, 1], dt.float32, isOutput=False)
    acy_in = nc.declare_dram_parameter("acy", [9, NPOS], dt.float32, isOutput=False)
    acx_in = nc.declare_dram_parameter("acx", [9, NPOS], dt.float32, isOutput=False)
    ah_in = nc.declare_dram_parameter("ah", [9, 1], dt.float32, isOutput=False)
    aw_in = nc.declare_dram_parameter("aw", [9, 1], dt.float32, isOutput=False)

    o_scores = nc.declare_dram_parameter("o_scores", [54, NPOS], dt.float32, isOutput=True)
    o_keys = nc.declare_dram_parameter("o_keys", [9, NPOS], dt.int32, isOutput=True)
    o_boxes = nc.declare_dram_parameter("o_boxes", [36, NPOS], dt.float32, isOutput=True)

    with TileContext(nc) as tc:
        with (
            tc.tile_pool(name="sb", bufs=1) as sb,
            tc.tile_pool(name="ps", bufs=1, space="PSUM") as ps,
        ):
            tx = sb.tile([128, 972], dt.float32, tag="tx")
            nc.sync.dma_start(out=tx[:], in_=x_in[:])
            tw1a = sb.tile([128, 4608], dt.float16, tag="tw1a")
            tw1b = sb.tile([128, 4608], dt.float16, tag="tw1b")
            tw1c = sb.tile([128, 4608], dt.float16, tag="tw1c")
            nc.sync.dma_start(out=tw1a[:], in_=w1a_in[:])
            nc.sync.dma_start(out=tw1b[:], in_=w1b_in[:])
            nc.sync.dma_start(out=tw1c[:], in_=w1c_in[:])
            tw2 = sb.tile([128, 324], dt.float16, tag="tw2")
            nc.sync.dma_start(out=tw2[:], in_=w2abc_in[:])
            tw2f = sb.tile([128, 108], dt.float32, tag="tw2f")
            nc.sync.dma_start(out=tw2f[:], in_=w2f_in[:])
            tb1 = sb.tile([128, 2], dt.float32, tag="tb1")
            nc.sync.dma_start(out=tb1[:], in_=b1_in[:])
            tb2 = sb.tile([54, 1], dt.float32, tag="tb2")
            nc.sync.dma_start(out=tb2[:], in_=b2_in[:])
            tacy = sb.tile([9, NPOS], dt.float32, tag="tacy")
            tacx = sb.tile([9, NPOS], dt.float32, tag="tacx")
            tah = sb.tile([9, 1], dt.float32, tag="tah")
            taw = sb.tile([9, 1], dt.float32, tag="taw")
            nc.sync.dma_start(out=tacy[:], in_=acy_in[:])
            nc.sync.dma_start(out=tacx[:], in_=acx_in[:])
            nc.sync.dma_start(out=tah[:], in_=ah_in[:])
            nc.sync.dma_start(out=taw[:], in_=aw_in[:])

            # ---- x fixed-point slices ----
            t8 = sb.tile([128, 972], dt.float32, tag="t8")
            nc.vector.tensor_scalar(t8[:], tx[:], float(2.0 ** 8), MAGIC, op.mult, op.add)
            nc.vector.tensor_scalar(t8[:], t8[:], MAGIC, None, op.subtract)
            xa16 = sb.tile([128, 972], dt.float16, tag="xa16")
            nc.vector.tensor_copy(xa16[:], t8[:])
            u = sb.tile([128, 972], dt.float32, tag="u")
            v = sb.tile([128, 972], dt.float32, tag="vv")
            nc.vector.tensor_scalar(v[:], tx[:], float(2.0 ** 19), None, op.mult)
            nc.vector.scalar_tensor_tensor(u[:], t8[:], float(-(2.0 ** 11)), v[:], op.mult, op.add)
            t19 = sb.tile([128, 972], dt.float32, tag="t19")
            nc.vector.tensor_scalar(t19[:], u[:], MAGIC, MAGIC, op.add, op.subtract)
            xb16 = sb.tile([128, 972], dt.float16, tag="xb16")
            nc.vector.tensor_copy(xb16[:], t19[:])
            nc.vector.tensor_scalar(v[:], tx[:], float(2.0 ** 30), None, op.mult)
            nc.vector.scalar_tensor_tensor(u[:], t8[:], float(-(2.0 ** 22)), v[:], op.mult, op.add)
            nc.vector.scalar_tensor_tensor(u[:], t19[:], float(-(2.0 ** 11)), u[:], op.mult, op.add)
            nc.vector.tensor_scalar(u[:], u[:], MAGIC, MAGIC, op.add, op.subtract)
            xc16 = sb.tile([128, 972], dt.float16, tag="xc16")
            nc.vector.tensor_copy(xc16[:], u[:])

            # 6-bank PSUM arena shared by stage 1 rounds and stage 2 streams
            pstile = ps.tile([128, 3072], dt.float32, tag="arena")
            psv = pstile[:].rearrange("p (c w) -> p c w", w=512)

            # ---- stage 1: W_eff slices ----
            # L1: per chunk 128 cols: [Wa(0:54) | pad | Wra(64:118) | pad]
            # L1b: per chunk 9 cols: Wrb (obj)
            # L2: per chunk 80 cols: [Wa(0:54) | pad | Wra-obj(64:73) | pad]
            # L3: per chunk 9 cols: Wa-obj
            L1 = sb.tile([128, 18 * 128], dt.float16, tag="L1")
            L1b = sb.tile([128, 18 * 9], dt.float16, tag="L1b")
            L2 = sb.tile([128, 18 * 80], dt.float16, tag="L2")
            L3 = sb.tile([128, 18 * 9], dt.float16, tag="L3")
            nc.vector.memset(L1[:], 0)
            nc.vector.memset(L2[:], 0)
            L1v = L1[:].rearrange("p (c w) -> p c w", w=128)
            L2v = L2[:].rearrange("p (c w) -> p c w", w=80)
            L1bv = L1b[:].rearrange("p (c w) -> p c w", w=9)
            L3v = L3[:].rearrange("p (c w) -> p c w", w=9)
            for rnd in range(3):
                for jj in range(6):
                    j = rnd * 6 + jj
                    base = jj * 512
                    for cc in range(2):
                        lhsA = tw1a[:, cc * 2304 + j * 128:cc * 2304 + (j + 1) * 128]
                        lhsB = tw1b[:, cc * 2304 + j * 128:cc * 2304 + (j + 1) * 128]
                        lhsC = tw1c[:, cc * 2304 + j * 128:cc * 2304 + (j + 1) * 128]
                        nc.tensor.matmul(pstile[:, base:base + 162], lhsA,
                                         tw2[:, cc * 162:cc * 162 + 162],
                                         start=(cc == 0), stop=False)
                        nc.tensor.matmul(pstile[:, base + 54:base + 162], lhsB,
                                         tw2[:, cc * 162:cc * 162 + 108],
                                         start=False, stop=False)
                        nc.tensor.matmul(pstile[:, base + 108:base + 162], lhsC,
                                         tw2[:, cc * 162:cc * 162 + 54],
                                         start=False, stop=(cc == 1))
                sl = slice(rnd * 6, (rnd + 1) * 6)
                T0 = sb.tile([128, 6, 54], dt.float32, tag="T0")
                nc.vector.tensor_scalar(T0[:], psv[:, 0:6, 0:54], float(2.0 ** -28), None, op.mult)
                WaF = sb.tile([128, 6, 54], dt.float32, tag="WaF")
                nc.vector.tensor_scalar(WaF[:], T0[:], float(2.0 ** 14), MAGIC, op.mult, op.add)
                nc.vector.tensor_scalar(WaF[:], WaF[:], MAGIC, None, op.subtract)
                nc.vector.tensor_copy(L1v[:, sl, 0:54], WaF[:])
                nc.scalar.copy(L2v[:, sl, 0:54], WaF[:])
                nc.scalar.copy(L3v[:, sl, 0:9], WaF[:, :, 0:9])
                Wr1 = sb.tile([128, 6, 54], dt.float32, tag="Wr1")
                nc.vector.scalar_tensor_tensor(Wr1[:], WaF[:], float(-(2.0 ** -14)), T0[:], op.mult, op.add)
                lo = sb.tile([128, 6, 54], dt.float32, tag="lo")
                nc.vector.tensor_scalar(lo[:], psv[:, 0:6, 54:108], float(2.0 ** -40), None, op.mult)
                nc.vector.scalar_tensor_tensor(lo[:], psv[:, 0:6, 108:162], float(2.0 ** -52), lo[:], op.mult, op.add)
                nc.vector.tensor_tensor(out=lo[:], in0=lo[:], in1=Wr1[:], op=op.add)
                WraF = sb.tile([128, 6, 54], dt.float32, tag="WraF")
                nc.vector.tensor_scalar(WraF[:], lo[:], float(2.0 ** 25), MAGIC, op.mult, op.add)
                nc.vector.tensor_scalar(WraF[:], WraF[:], MAGIC, None, op.subtract)
                nc.vector.tensor_copy(L1v[:, sl, 64:118], WraF[:])
                nc.scalar.copy(L2v[:, sl, 64:73], WraF[:, :, 0:9])
                Wrb = sb.tile([128, 6, 9], dt.float32, tag="Wrb")
                nc.vector.scalar_tensor_tensor(Wrb[:], WraF[:, :, 0:9], float(-(2.0 ** -25)),
                                               lo[:, :, 0:9], op.mult, op.add)
                nc.vector.tensor_scalar(Wrb[:], Wrb[:], float(2.0 ** 36), MAGIC, op.mult, op.add)
                nc.vector.tensor_scalar(Wrb[:], Wrb[:], MAGIC, None, op.subtract)
                nc.vector.tensor_copy(L1bv[:, sl, 0:9], Wrb[:])

            # ---- bias offs = w2^T b1 + b2 ----
            psO = pstile[0:54, 2048:2049]
            for cc in range(2):
                nc.tensor.matmul(psO, tw2f[:, cc * 54:(cc + 1) * 54], tb1[:, cc:cc + 1],
                                 start=(cc == 0), stop=(cc == 1))
            offs = sb.tile([54, 1], dt.float32, tag="offs")
            nc.vector.tensor_tensor(out=offs[:], in0=psO, in1=tb2[:], op=op.add)

            # ---- stage 2: 4 streams into 4 banks ----
            psA = pstile[0:118, 0:NPOS]          # xa x [Wa|pad|Wra]
            psAb = pstile[0:9, 512:512 + NPOS]   # xa x Wrb (obj)
            psB = pstile[0:73, 1024:1024 + NPOS]  # xb x [Wa|pad|Wra-obj]
            psC = pstile[0:9, 1536:1536 + NPOS]  # xc x Wa-obj
            xav = xa16[:].rearrange("p (c r q) -> p c r q", c=2, r=9)
            xbv = xb16[:].rearrange("p (c r q) -> p c r q", c=2, r=9)
            xcv = xc16[:].rearrange("p (c r q) -> p c r q", c=2, r=9)
            for tap in range(9):
                dy, dx = tap // 3, tap % 3
                for cc in range(2):
                    j = tap * 2 + cc
                    st = (j == 0)
                    sp = (j == 17)
                    ra = xav[:, cc, dy:dy + 7, dx:dx + 52]
                    rb = xbv[:, cc, dy:dy + 7, dx:dx + 52]
                    rc = xcv[:, cc, dy:dy + 7, dx:dx + 52]
                    nc.tensor.matmul(psA, L1[:, j * 128:j * 128 + 118], ra, start=st, stop=sp)
                    nc.tensor.matmul(psAb, L1b[:, j * 9:(j + 1) * 9], ra, start=st, stop=sp)
                    nc.tensor.matmul(psB, L2[:, j * 80:j * 80 + 73], rb, start=st, stop=sp)
                    nc.tensor.matmul(psC, L3[:, j * 9:(j + 1) * 9], rc, start=st, stop=sp)

            # ---- score combine ----
            c1 = sb.tile([54, NPOS], dt.float32, tag="c1")
            nc.vector.tensor_scalar(c1[:], pstile[64:118, 0:NPOS], float(2.0 ** -33), None, op.mult)
            Sres = sb.tile([54, NPOS], dt.float32, tag="Sres")
            nc.vector.scalar_tensor_tensor(Sres[:], pstile[0:54, 1024:1024 + NPOS],
                                           float(2.0 ** -33), c1[:], op.mult, op.add)
            e1 = sb.tile([9, NPOS], dt.float32, tag="e1")
            nc.vector.tensor_scalar(e1[:], psAb, float(2.0 ** -44), None, op.mult)
            nc.vector.scalar_tensor_tensor(e1[:], pstile[64:73, 1024:1024 + NPOS],
                                           float(2.0 ** -44), e1[:], op.mult, op.add)
            nc.vector.scalar_tensor_tensor(e1[:], psC, float(2.0 ** -44), e1[:], op.mult, op.add)
            nc.vector.tensor_tensor(out=Sres[0:9, :], in0=Sres[0:9, :], in1=e1[:], op=op.add)
            nc.vector.tensor_scalar(Sres[:], Sres[:], offs[:], None, op.add)
            S54 = sb.tile([54, NPOS], dt.float32, tag="S54")
            nc.vector.scalar_tensor_tensor(S54[:], pstile[0:54, 0:NPOS], float(2.0 ** -22),
                                           Sres[:], op.mult, op.add)
            nc.sync.dma_start(out=o_scores[:], in_=S54[:])

            # ---- int32 keys (before masking) ----
            kr = sb.tile([9, NPOS], dt.float32, tag="kr")
            nc.vector.tensor_scalar(kr[:], Sres[0:9, :], float(2.0 ** 31), MAGIC, op.mult, op.add)
            nc.vector.tensor_scalar(kr[:], kr[:], MAGIC, None, op.subtract)
            kri = sb.tile([9, NPOS], dt.int32, tag="kri")
            nc.vector.tensor_copy(kri[:], kr[:])
            k1 = sb.tile([9, NPOS], dt.int32, tag="k1")
            nc.vector.tensor_copy(k1[:], pstile[0:9, 0:NPOS])
            key = sb.tile([9, NPOS], dt.int32, tag="key")
            nc.vector.tensor_scalar(key[:], k1[:], 512, None, op.mult)
            nc.vector.tensor_tensor(out=key[:], in0=key[:], in1=kri[:], op=op.add)

            # ---- decode boxes ----
            Sl = []
            for d in range(4):
                t = sb.tile([9, NPOS], dt.float32, tag=f"Sl{d}")
                nc.sync.dma_start(out=t[:], in_=S54[18 + d * 9:27 + d * 9, :])
                Sl.append(t)
            cy = sb.tile([9, NPOS], dt.float32, tag="cy")
            cx = sb.tile([9, NPOS], dt.float32, tag="cx")
            nc.vector.scalar_tensor_tensor(cy[:], Sl[0][:], tah[:], tacy[:], op.mult, op.add)
            nc.vector.scalar_tensor_tensor(cx[:], Sl[1][:], taw[:], tacx[:], op.mult, op.add)
            eh = sb.tile([9, NPOS], dt.float32, tag="eh")
            ew = sb.tile([9, NPOS], dt.float32, tag="ew")
            for (srct, dstt) in ((Sl[2], eh), (Sl[3], ew)):
                nc.vector.tensor_scalar(dstt[:], srct[:], float(EXP_CO[0]), float(EXP_CO[1]),
                                        op.mult, op.add)
                for ci in range(2, 7):
                    nc.vector.tensor_tensor(out=dstt[:], in0=dstt[:], in1=srct[:], op=op.mult)
                    nc.vector.tensor_scalar(dstt[:], dstt[:], float(EXP_CO[ci]), None, op.add)
            nc.vector.tensor_scalar(eh[:], eh[:], tah[:], 0.5, op.mult, op.mult)
            nc.vector.tensor_scalar(ew[:], ew[:], taw[:], 0.5, op.mult, op.mult)
            bco = []
            for (ctr, half, sgn) in ((cy, eh, op.subtract), (cx, ew, op.subtract),
                                     (cy, eh, op.add), (cx, ew, op.add)):
                t = sb.tile([9, NPOS], dt.float32, tag=f"bco{len(bco)}")
                nc.vector.tensor_tensor(out=t[:], in0=ctr[:], in1=half[:], op=sgn)
                nc.vector.tensor_scalar(t[:], t[:], 0.0, float(IMG_W), op.max, op.min)
                nc.sync.dma_start(out=o_boxes[len(bco) * 9:(len(bco) + 1) * 9, :], in_=t[:])
                bco.append(t)
            hs = sb.tile([9, NPOS], dt.float32, tag="hs")
            ws = sb.tile([9, NPOS], dt.float32, tag="ws")
            nc.vector.tensor_tensor(out=hs[:], in0=bco[2][:], in1=bco[0][:], op=op.subtract)
            nc.vector.tensor_tensor(out=ws[:], in0=bco[3][:], in1=bco[1][:], op=op.subtract)
            nc.vector.tensor_scalar(hs[:], hs[:], 16.0, None, op.is_ge)
            nc.vector.tensor_scalar(ws[:], ws[:], 16.0, None, op.is_ge)
            nc.vector.tensor_tensor(out=hs[:], in0=hs[:], in1=ws[:], op=op.mult)
            vi = sb.tile([9, NPOS], dt.int32, tag="vi")
            nc.vector.tensor_copy(vi[:], hs[:])
            nc.vector.tensor_scalar(key[:], key[:], 2 ** 30, None, op.add)
            nc.vector.tensor_tensor(out=key[:], in0=key[:], in1=vi[:], op=op.mult)
            nc.vector.tensor_scalar(key[:], key[:], 2 ** 30, None, op.subtract)
            nc.sync.dma_start(out=o_keys[:], in_=key[:])

    nc.compile()
    _BUILD_CACHE["nc"] = nc
    return nc


def _host_inputs(x, conv1_w, conv1_b, reg_w, reg_b, cls_w, cls_b):
    x = np.asarray(x, f32)[0]
    w1 = np.asarray(conv1_w, f32)
    b1 = np.asarray(conv1_b, f32)
    wr = np.asarray(reg_w, f32)[:, :, 0, 0]
    br = np.asarray(reg_b, f32)
    wc = np.asarray(cls_w, f32)[:, :, 0, 0]
    bc = np.asarray(cls_b, f32)

    w2_all = np.concatenate([wr, wc], 0)
    b2_all = np.concatenate([br, bc], 0)
    w2p = np.ascontiguousarray(w2_all[CHAN_PERM].T.astype(f32))
    b2p = b2_all[CHAN_PERM].astype(f32)

    w1p = np.ascontiguousarray(w1.transpose(0, 2, 3, 1).reshape(256, 2304).astype(f32))
    w1a, w1b, w1c = _slice3(w1p, 14, 26, 38)
    w2a, w2b, w2c = _slice3(w2p, 14, 26, 38)

    def two_chunk(m):
        F = m.shape[1]
        out = np.empty((128, 2 * F), m.dtype)
        out[:, :F] = m[:128]
        out[:, F:] = m[128:]
        return out

    w1a_h = two_chunk(w1a).astype(np.float16)
    w1b_h = two_chunk(w1b).astype(np.float16)
    w1c_h = two_chunk(w1c).astype(np.float16)
    w2abc_h = two_chunk(np.concatenate([w2a, w2b, w2c], 1)).astype(np.float16)
    w2f_h = two_chunk(w2p).astype(f32)
    b1_h = np.ascontiguousarray(b1.reshape(2, 128).T.astype(f32))
    b2_h = b2p.reshape(54, 1)

    vals, ahv, awv = _make_anchor_geom()
    ah_h = ahv.reshape(9, 1).astype(f32)
    aw_h = awv.reshape(9, 1).astype(f32)

    xpad = np.zeros((256, 58, 54), f32)
    xpad[:, 1:53, 1:53] = x
    valsf = vals.astype(f32)
    in_maps = []
    for c in range(N_CORES):
        r0 = c * R
        stripe = xpad[:, r0:r0 + 9, :]
        x_h = two_chunk(np.ascontiguousarray(stripe.reshape(256, 9 * 54)))
        acy = np.zeros((9, NPOS), f32)
        acx = np.zeros((9, NPOS), f32)
        for rr in range(R):
            gr = r0 + rr
            acy[:, rr * 52:(rr + 1) * 52] = valsf[gr] if gr < FEAT_W else f32(0)
            acx[:, rr * 52:(rr + 1) * 52] = valsf[None, :FEAT_W]
        in_maps.append({
            "x_in": x_h, "w1a": w1a_h, "w1b": w1b_h, "w1c": w1c_h,
            "w2abc": w2abc_h, "w2f": w2f_h, "b1": b1_h, "b2": b2_h,
            "acy": acy, "acx": acx, "ah": ah_h, "aw": aw_h,
        })
    return in_maps


def _host_nms(keys, boxes):
    order = np.argsort(-keys.astype(np.int64), kind="stable")[:PRE_NMS]
    b = boxes[order]
    areas = (b[:, 3] - b[:, 1] + f32(1)) * (b[:, 2] - b[:, 0] + f32(1))
    N = len(order)
    idx = np.arange(N)
    supp = np.zeros(N, bool)
    kept = []
    for i in range(N):
        if supp[i]:
            continue
        kept.append(i)
        if len(kept) == POST_NMS:
            break
        bi = b[i]
        yy1 = np.maximum(bi[0], b[:, 0])
        xx1 = np.maximum(bi[1], b[:, 1])
        yy2 = np.minimum(bi[2], b[:, 2])
        xx2 = np.minimum(bi[3], b[:, 3])
        inter = np.maximum(f32(0), xx2 - xx1 + f32(1)) * np.maximum(f32(0), yy2 - yy1 + f32(1))
        iou = inter / (areas[i] + areas - inter)
        supp |= (iou > f32(0.7)) & (idx > i)
    out = np.zeros((POST_NMS, 4), f32)
    k = np.array(kept[:POST_NMS])
    out[:len(k)] = b[k]
    return out


TRACE = False
LAST_RESULT = [None]


def kernel(x, conv1_w, conv1_b, reg_w, reg_b, cls_w, cls_b):
    from concourse.bass_utils import run_bass_kernel_spmd
    nc = _build()
    in_maps = _host_inputs(x, conv1_w, conv1_b, reg_w, reg_b, cls_w, cls_b)
    res = run_bass_kernel_spmd(nc, in_maps, list(range(N_CORES)), trace=TRACE)
    LAST_RESULT[0] = res

    scores = np.zeros((54, 56 * 52), f32)
    keys = np.zeros((9, 56 * 52), np.int32)
    boxes = np.zeros((36, 56 * 52), f32)
    for c in range(N_CORES):
        r = res.results[c]
        scores[:, c * NPOS:(c + 1) * NPOS] = r["o_scores"]
        keys[:, c * NPOS:(c + 1) * NPOS] = np.asarray(r["o_keys"]).view(np.int32)
        boxes[:, c * NPOS:(c + 1) * NPOS] = r["o_boxes"]
    npos = FEAT_W * FEAT_W
    scores = scores[:, :npos]
    keys = keys[:, :npos]
    boxes = boxes[:, :npos]

    locs = np.empty((npos, NA, 4), f32)
    for d in range(4):
        locs[:, :, d] = scores[18 + d * 9:27 + d * 9, :].T
    locs = locs.reshape(1, npos * NA, 4)
    cls_scores = np.empty((npos, NA, 2), f32)
    cls_scores[:, :, 0] = scores[9:18, :].T
    cls_scores[:, :, 1] = scores[0:9, :].T
    cls_scores = cls_scores.reshape(1, npos * NA, 2)
    obj = np.ascontiguousarray(cls_scores[:, :, 1].reshape(1, npos * NA))

    kflat = keys.T.reshape(-1)
    bflat = np.stack([boxes[0:9].T.reshape(-1), boxes[9:18].T.reshape(-1),
                      boxes[18:27].T.reshape(-1), boxes[27:36].T.reshape(-1)], 1)
    rois = _host_nms(kflat, bflat)
    return (rois, locs, cls_scores, obj, cls_scores)


# revision 12
# speedup vs baseline: 1.0397x; 1.0397x over previous
# Trainium2 Bass kernel for nn_RPNmodel (RPN conv head + proposal decode + NMS).
#
# Device (8 NeuronCores, SPMD, feature rows sharded 7 rows/core):
#   - effective-weight contraction W_eff[k,o] = sum_c w1[c,k]*w2[o,c] on the PE,
#     using fp16 fixed-point slices with exact integer accumulation in PSUM
#   - conv-as-GEMM over the 9 taps: scores + locs for the core's positions (PE)
#   - box decode (polynomial exp), validity, int32 sort keys (DVE)
# Host: shards/pads inputs, slices weights into fixed-point fp16 words (a
# lossless re-encoding), gathers per-core outputs, and applies the greedy NMS
# ordering on the device-computed keys/boxes to emit rois.
#
# Precision: objectness scores are produced as exact-integer hi words (grid
# 2^-22) plus fp32 residuals => |score - exact| ~1e-9, so the descending key
# order reproduces the fp32 reference ordering wherever it is determined.
import numpy as np

FEAT_W = 52
IMG_W = 210.0
SUB = 4
NA = 9
N_CORES = 8
R = 7                  # feature rows per core (8*7 = 56 >= 52, tail zero-padded)
NPOS = R * FEAT_W      # 364 positions per core
PRE_NMS = 12000
POST_NMS = 2000

f32 = np.float32
MAGIC = float(3 * 2 ** 22)

# device channel order (54 rows), all 32-alignment-friendly groups:
#  0:9 obj | 9:18 cls-even | 18:27 l0 | 27:36 l1 | 36:45 l2 | 45:54 l3
def _chan_perm():
    reg = [a * 4 + d for d in range(4) for a in range(NA)]
    cls_even = [36 + 2 * a for a in range(NA)]
    cls_odd = [36 + 2 * a + 1 for a in range(NA)]
    return np.array(cls_odd + cls_even + reg, np.int64)

CHAN_PERM = _chan_perm()

def _magic_round(x, scale=1.0):
    m = f32(MAGIC)
    t = (x.astype(f32) * f32(scale) + m).astype(f32)
    return (t - m).astype(f32)

def _slice3(w, g1, g2, g3):
    a = _magic_round(w, 2.0 ** g1)
    b = _magic_round(w * f32(2.0 ** g2) - a * f32(2.0 ** (g2 - g1)), 1.0)
    c = _magic_round(w * f32(2.0 ** g3) - a * f32(2.0 ** (g3 - g1)) - b * f32(2.0 ** (g3 - g2)), 1.0)
    return a, b, c

def _make_anchor_geom():
    vals = (np.arange(SUB, (FEAT_W + 1) * SUB, SUB) - SUB // 2).astype(np.float64)
    ratios = np.array([0.5, 1.0, 2.0])
    scales = np.array([4.0, 8.0, 16.0])
    h = (SUB * scales[None, :] * np.sqrt(ratios[:, None])).reshape(-1)
    w = (SUB * scales[None, :] * np.sqrt(1.0 / ratios[:, None])).reshape(-1)
    return vals, h.astype(f32), w.astype(f32)

def _exp_poly():
    t = np.linspace(-0.37, 0.37, 4001)
    return np.polyfit(t, np.exp(t), 6).astype(np.float64)

EXP_CO = _exp_poly()

_BUILD_CACHE = {}

def _build():
    if "nc" in _BUILD_CACHE:
        return _BUILD_CACHE["nc"]
    import concourse.bacc as bacc
    import concourse.mybir as mybir
    from concourse.tile import TileContext

    dt = mybir.dt
    op = mybir.AluOpType
    nc = bacc.Bacc("TRN2", target_bir_lowering=False, debug=False, num_devices=N_CORES)

    x_in = nc.declare_dram_parameter("x_in", [128, 2 * 9 * 54], dt.float32, isOutput=False)
    w1a_in = nc.declare_dram_parameter("w1a", [128, 2 * 2304], dt.float16, isOutput=False)
    w1b_in = nc.declare_dram_parameter("w1b", [128, 2 * 2304], dt.float16, isOutput=False)
    w1c_in = nc.declare_dram_parameter("w1c", [128, 2 * 2304], dt.float16, isOutput=False)
    w2abc_in = nc.declare_dram_parameter("w2abc", [128, 2 * 162], dt.float16, isOutput=False)
    w2f_in = nc.declare_dram_parameter("w2f", [128, 2 * 54], dt.float32, isOutput=False)
    b1_in = nc.declare_dram_parameter("b1", [128, 2], dt.float32, isOutput=False)
    b2_in = nc.declare_dram_parameter("b2", [54, 1], dt.float32, isOutput=False)
    acy_in = nc.declare_dram_parameter("acy", [9, NPOS], dt.float32, isOutput=False)
    acx_in = nc.declare_dram_parameter("acx", [9, NPOS], dt.float32, isOutput=False)
    ah_in = nc.declare_dram_parameter("ah", [9, 1], dt.float32, isOutput=False)
    aw_in = nc.declare_dram_parameter("aw", [9, 1], dt.float32, isOutput=False)

    o_scores = nc.declare_dram_parameter("o_scores", [54, NPOS], dt.float32, isOutput=True)
    o_keys = nc.declare_dram_parameter("o_keys", [9, NPOS], dt.int32, isOutput=True)
    o_boxes = nc.declare_dram_parameter("o_boxes", [36, NPOS], dt.float32, isOutput=True)

    with TileContext(nc) as tc:
        with (
            tc.tile_pool(name="sb", bufs=1) as sb,
            tc.tile_pool(name="ps", bufs=1, space="PSUM") as ps,
        ):
            tx = sb.tile([128, 972], dt.float32, tag="tx")
            nc.sync.dma_start(out=tx[:], in_=x_in[:])
            tw2 = sb.tile([128, 324], dt.float16, tag="tw2")
            nc.sync.dma_start(out=tw2[:], in_=w2abc_in[:])
            # weights DMA'd round-major so stage-1 round 0 starts after 1/3 lands
            tw1a = sb.tile([128, 4608], dt.float16, tag="tw1a")
            tw1b = sb.tile([128, 4608], dt.float16, tag="tw1b")
            tw1c = sb.tile([128, 4608], dt.float16, tag="tw1c")
            for rnd in range(3):
                for t_, p_ in ((tw1a, w1a_in), (tw1b, w1b_in), (tw1c, w1c_in)):
                    for cc in range(2):
                        s0 = cc * 2304 + rnd * 768
                        nc.sync.dma_start(out=t_[:, s0:s0 + 768], in_=p_[:, s0:s0 + 768])
            tw2f = sb.tile([128, 108], dt.float32, tag="tw2f")
            nc.sync.dma_start(out=tw2f[:], in_=w2f_in[:])
            tb1 = sb.tile([128, 2], dt.float32, tag="tb1")
            nc.sync.dma_start(out=tb1[:], in_=b1_in[:])
            tb2 = sb.tile([54, 1], dt.float32, tag="tb2")
            nc.sync.dma_start(out=tb2[:], in_=b2_in[:])
            tacy = sb.tile([9, NPOS], dt.float32, tag="tacy")
            tacx = sb.tile([9, NPOS], dt.float32, tag="tacx")
            tah = sb.tile([9, 1], dt.float32, tag="tah")
            taw = sb.tile([9, 1], dt.float32, tag="taw")
            nc.sync.dma_start(out=tacy[:], in_=acy_in[:])
            nc.sync.dma_start(out=tacx[:], in_=acx_in[:])
            nc.sync.dma_start(out=tah[:], in_=ah_in[:])
            nc.sync.dma_start(out=taw[:], in_=aw_in[:])

            # ---- x fixed-point slices ----
            t8 = sb.tile([128, 972], dt.float32, tag="t8")
            nc.vector.tensor_scalar(t8[:], tx[:], float(2.0 ** 8), MAGIC, op.mult, op.add)
            nc.vector.tensor_scalar(t8[:], t8[:], MAGIC, None, op.subtract)
            xa16 = sb.tile([128, 972], dt.float16, tag="xa16")
            nc.vector.tensor_copy(xa16[:], t8[:])
            u = sb.tile([128, 972], dt.float32, tag="u")
            v = sb.tile([128, 972], dt.float32, tag="vv")
            nc.vector.tensor_scalar(v[:], tx[:], float(2.0 ** 19), None, op.mult)
            nc.vector.scalar_tensor_tensor(u[:], t8[:], float(-(2.0 ** 11)), v[:], op.mult, op.add)
            t19 = sb.tile([128, 972], dt.float32, tag="t19")
            nc.vector.tensor_scalar(t19[:], u[:], MAGIC, MAGIC, op.add, op.subtract)
            xb16 = sb.tile([128, 972], dt.float16, tag="xb16")
            nc.vector.tensor_copy(xb16[:], t19[:])
            nc.vector.tensor_scalar(v[:], tx[:], float(2.0 ** 30), None, op.mult)
            nc.vector.scalar_tensor_tensor(u[:], t8[:], float(-(2.0 ** 22)), v[:], op.mult, op.add)
            nc.vector.scalar_tensor_tensor(u[:], t19[:], float(-(2.0 ** 11)), u[:], op.mult, op.add)
            nc.vector.tensor_scalar(u[:], u[:], MAGIC, MAGIC, op.add, op.subtract)
            xc16 = sb.tile([128, 972], dt.float16, tag="xc16")
            nc.vector.tensor_copy(xc16[:], u[:])

            # 6-bank PSUM arena shared by stage 1 rounds and stage 2 streams
            pstile = ps.tile([128, 3072], dt.float32, tag="arena")
            psv = pstile[:].rearrange("p (c w) -> p c w", w=512)

            # ---- stage 1: W_eff slices ----
            # L1: per chunk 128 cols: [Wa(0:54) | pad | Wra(64:118) | pad]
            # L1b: per chunk 9 cols: Wrb (obj)
            # L2: per chunk 80 cols: [Wa(0:54) | pad | Wra-obj(64:73) | pad]
            # L3: per chunk 9 cols: Wa-obj
            L1 = sb.tile([128, 18 * 128], dt.float16, tag="L1")
            L1b = sb.tile([128, 18 * 9], dt.float16, tag="L1b")
            L2 = sb.tile([128, 18 * 80], dt.float16, tag="L2")
            L3 = sb.tile([128, 18 * 9], dt.float16, tag="L3")
            nc.vector.memset(L1[:], 0)
            nc.vector.memset(L2[:], 0)
            L1v = L1[:].rearrange("p (c w) -> p c w", w=128)
            L2v = L2[:].rearrange("p (c w) -> p c w", w=80)
            L1bv = L1b[:].rearrange("p (c w) -> p c w", w=9)
            L3v = L3[:].rearrange("p (c w) -> p c w", w=9)
            for rnd in range(3):
                for jj in range(6):
                    j = rnd * 6 + jj
                    base = jj * 512
                    for cc in range(2):
                        lhsA = tw1a[:, cc * 2304 + j * 128:cc * 2304 + (j + 1) * 128]
                        lhsB = tw1b[:, cc * 2304 + j * 128:cc * 2304 + (j + 1) * 128]
                        lhsC = tw1c[:, cc * 2304 + j * 128:cc * 2304 + (j + 1) * 128]
                        nc.tensor.matmul(pstile[:, base:base + 162], lhsA,
                                         tw2[:, cc * 162:cc * 162 + 162],
                                         start=(cc == 0), stop=False)
                        nc.tensor.matmul(pstile[:, base + 54:base + 162], lhsB,
                                         tw2[:, cc * 162:cc * 162 + 108],
                                         start=False, stop=False)
                        nc.tensor.matmul(pstile[:, base + 108:base + 162], lhsC,
                                         tw2[:, cc * 162:cc * 162 + 54],
                                         start=False, stop=(cc == 1))
                sl = slice(rnd * 6, (rnd + 1) * 6)
                hv = psv[:, 0:6, :]
                T0 = sb.tile([128, 6, 54], dt.float32, tag="T0")
                nc.vector.tensor_scalar(T0[:], hv[:, :, 0:54], float(2.0 ** -28), None, op.mult)
                WaF = sb.tile([128, 6, 54], dt.float32, tag="WaF")
                nc.vector.tensor_scalar(WaF[:], T0[:], float(2.0 ** 14), MAGIC, op.mult, op.add)
                nc.vector.tensor_scalar(WaF[:], WaF[:], MAGIC, None, op.subtract)
                nc.vector.tensor_copy(L1v[:, sl, 0:54], WaF[:])
                nc.scalar.copy(L2v[:, sl, 0:54], WaF[:])
                nc.scalar.copy(L3v[:, sl, 0:9], WaF[:, :, 0:9])
                Wr1 = sb.tile([128, 6, 54], dt.float32, tag="Wr1")
                nc.vector.scalar_tensor_tensor(Wr1[:], WaF[:], float(-(2.0 ** -14)), T0[:], op.mult, op.add)
                lo = sb.tile([128, 6, 54], dt.float32, tag="lo")
                nc.vector.tensor_scalar(lo[:], hv[:, :, 54:108], float(2.0 ** -40), None, op.mult)
                nc.vector.scalar_tensor_tensor(lo[:], hv[:, :, 108:162], float(2.0 ** -52), lo[:], op.mult, op.add)
                nc.vector.tensor_tensor(out=lo[:], in0=lo[:], in1=Wr1[:], op=op.add)
                WraF = sb.tile([128, 6, 54], dt.float32, tag="WraF")
                nc.vector.tensor_scalar(WraF[:], lo[:], float(2.0 ** 25), MAGIC, op.mult, op.add)
                nc.vector.tensor_scalar(WraF[:], WraF[:], MAGIC, None, op.subtract)
                nc.vector.tensor_copy(L1v[:, sl, 64:118], WraF[:])
                nc.scalar.copy(L2v[:, sl, 64:73], WraF[:, :, 0:9])
                Wrb = sb.tile([128, 6, 9], dt.float32, tag="Wrb")
                nc.vector.scalar_tensor_tensor(Wrb[:], WraF[:, :, 0:9], float(-(2.0 ** -25)),
                                               lo[:, :, 0:9], op.mult, op.add)
                nc.vector.tensor_scalar(Wrb[:], Wrb[:], float(2.0 ** 36), MAGIC, op.mult, op.add)
                nc.vector.tensor_scalar(Wrb[:], Wrb[:], MAGIC, None, op.subtract)
                nc.vector.tensor_copy(L1bv[:, sl, 0:9], Wrb[:])

            # ---- bias offs = w2^T b1 + b2 ----
            psO = pstile[0:54, 2048:2049]
            for cc in range(2):
                nc.tensor.matmul(psO, tw2f[:, cc * 54:(cc + 1) * 54], tb1[:, cc:cc + 1],
                                 start=(cc == 0), stop=(cc == 1))
            offs = sb.tile([54, 1], dt.float32, tag="offs")
            nc.vector.tensor_tensor(out=offs[:], in0=psO, in1=tb2[:], op=op.add)

            # ---- stage 2: 4 streams into 4 banks ----
            psA = pstile[0:118, 0:NPOS]          # xa x [Wa|pad|Wra]
            psAb = pstile[0:9, 512:512 + NPOS]   # xa x Wrb (obj)
            psB = pstile[0:73, 1024:1024 + NPOS]  # xb x [Wa|pad|Wra-obj]
            psC = pstile[0:9, 1536:1536 + NPOS]  # xc x Wa-obj
            xav = xa16[:].rearrange("p (c r q) -> p c r q", c=2, r=9)
            xbv = xb16[:].rearrange("p (c r q) -> p c r q", c=2, r=9)
            xcv = xc16[:].rearrange("p (c r q) -> p c r q", c=2, r=9)
            for tap in range(9):
                dy, dx = tap // 3, tap % 3
                for cc in range(2):
                    j = tap * 2 + cc
                    st = (j == 0)
                    sp = (j == 17)
                    ra = xav[:, cc, dy:dy + 7, dx:dx + 52]
                    rb = xbv[:, cc, dy:dy + 7, dx:dx + 52]
                    rc = xcv[:, cc, dy:dy + 7, dx:dx + 52]
                    nc.tensor.matmul(psA, L1[:, j * 128:j * 128 + 118], ra, start=st, stop=sp)
                    nc.tensor.matmul(psAb, L1b[:, j * 9:(j + 1) * 9], ra, start=st, stop=sp)
                    nc.tensor.matmul(psB, L2[:, j * 80:j * 80 + 73], rb, start=st, stop=sp)
                    nc.tensor.matmul(psC, L3[:, j * 9:(j + 1) * 9], rc, start=st, stop=sp)

            # ---- score combine ----
            c1 = sb.tile([54, NPOS], dt.float32, tag="c1")
            nc.vector.tensor_scalar(c1[:], pstile[64:118, 0:NPOS], float(2.0 ** -33), None, op.mult)
            Sres = sb.tile([54, NPOS], dt.float32, tag="Sres")
            nc.vector.scalar_tensor_tensor(Sres[:], pstile[0:54, 1024:1024 + NPOS],
                                           float(2.0 ** -33), c1[:], op.mult, op.add)
            e1 = sb.tile([9, NPOS], dt.float32, tag="e1")
            nc.vector.tensor_scalar(e1[:], psAb, float(2.0 ** -44), None, op.mult)
            nc.vector.scalar_tensor_tensor(e1[:], pstile[64:73, 1024:1024 + NPOS],
                                           float(2.0 ** -44), e1[:], op.mult, op.add)
            nc.vector.scalar_tensor_tensor(e1[:], psC, float(2.0 ** -44), e1[:], op.mult, op.add)
            nc.vector.tensor_tensor(out=Sres[0:9, :], in0=Sres[0:9, :], in1=e1[:], op=op.add)
            nc.vector.tensor_scalar(Sres[:], Sres[:], offs[:], None, op.add)
            S54 = sb.tile([54, NPOS], dt.float32, tag="S54")
            nc.vector.scalar_tensor_tensor(S54[:], pstile[0:54, 0:NPOS], float(2.0 ** -22),
                                           Sres[:], op.mult, op.add)
            nc.sync.dma_start(out=o_scores[:], in_=S54[:])

            # ---- int32 keys (before masking) ----
            kr = sb.tile([9, NPOS], dt.float32, tag="kr")
            nc.vector.tensor_scalar(kr[:], Sres[0:9, :], float(2.0 ** 31), MAGIC, op.mult, op.add)
            nc.vector.tensor_scalar(kr[:], kr[:], MAGIC, None, op.subtract)
            kri = sb.tile([9, NPOS], dt.int32, tag="kri")
            nc.vector.tensor_copy(kri[:], kr[:])
            k1 = sb.tile([9, NPOS], dt.int32, tag="k1")
            nc.vector.tensor_copy(k1[:], pstile[0:9, 0:NPOS])
            key = sb.tile([9, NPOS], dt.int32, tag="key")
            nc.vector.tensor_scalar(key[:], k1[:], 512, None, op.mult)
            nc.vector.tensor_tensor(out=key[:], in0=key[:], in1=kri[:], op=op.add)

            # ---- decode boxes ----
            Sl = []
            for d in range(4):
                t = sb.tile([9, NPOS], dt.float32, tag=f"Sl{d}")
                nc.sync.dma_start(out=t[:], in_=S54[18 + d * 9:27 + d * 9, :])
                Sl.append(t)
            cy = sb.tile([9, NPOS], dt.float32, tag="cy")
            cx = sb.tile([9, NPOS], dt.float32, tag="cx")
            nc.vector.scalar_tensor_tensor(cy[:], Sl[0][:], tah[:], tacy[:], op.mult, op.add)
            nc.vector.scalar_tensor_tensor(cx[:], Sl[1][:], taw[:], tacx[:], op.mult, op.add)
            eh = sb.tile([9, NPOS], dt.float32, tag="eh")
            ew = sb.tile([9, NPOS], dt.float32, tag="ew")
            for (srct, dstt) in ((Sl[2], eh), (Sl[3], ew)):
                nc.vector.tensor_scalar(dstt[:], srct[:], float(EXP_CO[0]), float(EXP_CO[1]),
                                        op.mult, op.add)
                for ci in range(2, 7):
                    nc.vector.tensor_tensor(out=dstt[:], in0=dstt[:], in1=srct[:], op=op.mult)
                    nc.vector.tensor_scalar(dstt[:], dstt[:], float(EXP_CO[ci]), None, op.add)
            nc.vector.tensor_scalar(eh[:], eh[:], tah[:], 0.5, op.mult, op.mult)
            nc.vector.tensor_scalar(ew[:], ew[:], taw[:], 0.5, op.mult, op.mult)
            bco = []
            for (ctr, half, sgn) in ((cy, eh, op.subtract), (cx, ew, op.subtract),
                                     (cy, eh, op.add), (cx, ew, op.add)):
                t = sb.tile([9, NPOS], dt.float32, tag=f"bco{len(bco)}")
                nc.vector.tensor_tensor(out=t[:], in0=ctr[:], in1=half[:], op=sgn)
                nc.vector.tensor_scalar(t[:], t[:], 0.0, float(IMG_W), op.max, op.min)
                nc.sync.dma_start(out=o_boxes[len(bco) * 9:(len(bco) + 1) * 9, :], in_=t[:])
                bco.append(t)
            hs = sb.tile([9, NPOS], dt.float32, tag="hs")
            ws = sb.tile([9, NPOS], dt.float32, tag="ws")
            nc.vector.tensor_tensor(out=hs[:], in0=bco[2][:], in1=bco[0][:], op=op.subtract)
            nc.vector.tensor_tensor(out=ws[:], in0=bco[3][:], in1=bco[1][:], op=op.subtract)
            nc.vector.tensor_scalar(hs[:], hs[:], 16.0, None, op.is_ge)
            nc.vector.tensor_scalar(ws[:], ws[:], 16.0, None, op.is_ge)
            nc.vector.tensor_tensor(out=hs[:], in0=hs[:], in1=ws[:], op=op.mult)
            vi = sb.tile([9, NPOS], dt.int32, tag="vi")
            nc.vector.tensor_copy(vi[:], hs[:])
            nc.vector.tensor_scalar(key[:], key[:], 2 ** 30, None, op.add)
            nc.vector.tensor_tensor(out=key[:], in0=key[:], in1=vi[:], op=op.mult)
            nc.vector.tensor_scalar(key[:], key[:], 2 ** 30, None, op.subtract)
            nc.sync.dma_start(out=o_keys[:], in_=key[:])

    nc.compile()
    _BUILD_CACHE["nc"] = nc
    return nc


def _host_inputs(x, conv1_w, conv1_b, reg_w, reg_b, cls_w, cls_b):
    x = np.asarray(x, f32)[0]
    w1 = np.asarray(conv1_w, f32)
    b1 = np.asarray(conv1_b, f32)
    wr = np.asarray(reg_w, f32)[:, :, 0, 0]
    br = np.asarray(reg_b, f32)
    wc = np.asarray(cls_w, f32)[:, :, 0, 0]
    bc = np.asarray(cls_b, f32)

    w2_all = np.concatenate([wr, wc], 0)
    b2_all = np.concatenate([br, bc], 0)
    w2p = np.ascontiguousarray(w2_all[CHAN_PERM].T.astype(f32))
    b2p = b2_all[CHAN_PERM].astype(f32)

    w1p = np.ascontiguousarray(w1.transpose(0, 2, 3, 1).reshape(256, 2304).astype(f32))
    w1a, w1b, w1c = _slice3(w1p, 14, 26, 38)
    w2a, w2b, w2c = _slice3(w2p, 14, 26, 38)

    def two_chunk(m):
        F = m.shape[1]
        out = np.empty((128, 2 * F), m.dtype)
        out[:, :F] = m[:128]
        out[:, F:] = m[128:]
        return out

    w1a_h = two_chunk(w1a).astype(np.float16)
    w1b_h = two_chunk(w1b).astype(np.float16)
    w1c_h = two_chunk(w1c).astype(np.float16)
    w2abc_h = two_chunk(np.concatenate([w2a, w2b, w2c], 1)).astype(np.float16)
    w2f_h = two_chunk(w2p).astype(f32)
    b1_h = np.ascontiguousarray(b1.reshape(2, 128).T.astype(f32))
    b2_h = b2p.reshape(54, 1)

    vals, ahv, awv = _make_anchor_geom()
    ah_h = ahv.reshape(9, 1).astype(f32)
    aw_h = awv.reshape(9, 1).astype(f32)

    xpad = np.zeros((256, 58, 54), f32)
    xpad[:, 1:53, 1:53] = x
    valsf = vals.astype(f32)
    in_maps = []
    for c in range(N_CORES):
        r0 = c * R
        stripe = xpad[:, r0:r0 + 9, :]
        x_h = two_chunk(np.ascontiguousarray(stripe.reshape(256, 9 * 54)))
        acy = np.zeros((9, NPOS), f32)
        acx = np.zeros((9, NPOS), f32)
        for rr in range(R):
            gr = r0 + rr
            acy[:, rr * 52:(rr + 1) * 52] = valsf[gr] if gr < FEAT_W else f32(0)
            acx[:, rr * 52:(rr + 1) * 52] = valsf[None, :FEAT_W]
        in_maps.append({
            "x_in": x_h, "w1a": w1a_h, "w1b": w1b_h, "w1c": w1c_h,
            "w2abc": w2abc_h, "w2f": w2f_h, "b1": b1_h, "b2": b2_h,
            "acy": acy, "acx": acx, "ah": ah_h, "aw": aw_h,
        })
    return in_maps


def _host_nms(keys, boxes):
    order = np.argsort(-keys.astype(np.int64), kind="stable")[:PRE_NMS]
    b = boxes[order]
    areas = (b[:, 3] - b[:, 1] + f32(1)) * (b[:, 2] - b[:, 0] + f32(1))
    N = len(order)
    idx = np.arange(N)
    supp = np.zeros(N, bool)
    kept = []
    for i in range(N):
        if supp[i]:
            continue
        kept.append(i)
        if len(kept) == POST_NMS:
            break
        bi = b[i]
        yy1 = np.maximum(bi[0], b[:, 0])
        xx1 = np.maximum(bi[1], b[:, 1])
        yy2 = np.minimum(bi[2], b[:, 2])
        xx2 = np.minimum(bi[3], b[:, 3])
        inter = np.maximum(f32(0), xx2 - xx1 + f32(1)) * np.maximum(f32(0), yy2 - yy1 + f32(1))
        iou = inter / (areas[i] + areas - inter)
        supp |= (iou > f32(0.7)) & (idx > i)
    out = np.zeros((POST_NMS, 4), f32)
    k = np.array(kept[:POST_NMS])
    out[:len(k)] = b[k]
    return out


TRACE = False
LAST_RESULT = [None]


def kernel(x, conv1_w, conv1_b, reg_w, reg_b, cls_w, cls_b):
    from concourse.bass_utils import run_bass_kernel_spmd
    nc = _build()
    in_maps = _host_inputs(x, conv1_w, conv1_b, reg_w, reg_b, cls_w, cls_b)
    res = run_bass_kernel_spmd(nc, in_maps, list(range(N_CORES)), trace=TRACE)
    LAST_RESULT[0] = res

    scores = np.zeros((54, 56 * 52), f32)
    keys = np.zeros((9, 56 * 52), np.int32)
    boxes = np.zeros((36, 56 * 52), f32)
    for c in range(N_CORES):
        r = res.results[c]
        scores[:, c * NPOS:(c + 1) * NPOS] = r["o_scores"]
        keys[:, c * NPOS:(c + 1) * NPOS] = np.asarray(r["o_keys"]).view(np.int32)
        boxes[:, c * NPOS:(c + 1) * NPOS] = r["o_boxes"]
    npos = FEAT_W * FEAT_W
    scores = scores[:, :npos]
    keys = keys[:, :npos]
    boxes = boxes[:, :npos]

    locs = np.empty((npos, NA, 4), f32)
    for d in range(4):
        locs[:, :, d] = scores[18 + d * 9:27 + d * 9, :].T
    locs = locs.reshape(1, npos * NA, 4)
    cls_scores = np.empty((npos, NA, 2), f32)
    cls_scores[:, :, 0] = scores[9:18, :].T
    cls_scores[:, :, 1] = scores[0:9, :].T
    cls_scores = cls_scores.reshape(1, npos * NA, 2)
    obj = np.ascontiguousarray(cls_scores[:, :, 1].reshape(1, npos * NA))

    kflat = keys.T.reshape(-1)
    bflat = np.stack([boxes[0:9].T.reshape(-1), boxes[9:18].T.reshape(-1),
                      boxes[18:27].T.reshape(-1), boxes[27:36].T.reshape(-1)], 1)
    rois = _host_nms(kflat, bflat)
    return (rois, locs, cls_scores, obj, cls_scores)


# revision 13
# speedup vs baseline: 1.0643x; 1.0236x over previous
# Trainium2 Bass kernel for nn_RPNmodel (RPN conv head + proposal decode + NMS).
#
# Device (8 NeuronCores, SPMD, feature rows sharded 7 rows/core):
#   - effective-weight contraction W_eff[k,o] = sum_c w1[c,k]*w2[o,c] on the PE,
#     using fp16 fixed-point slices with exact integer accumulation in PSUM
#   - conv-as-GEMM over the 9 taps: scores + locs for the core's positions (PE)
#   - box decode (polynomial exp), validity, int32 sort keys (DVE)
# Host: shards/pads inputs, slices weights into fixed-point fp16 words (a
# lossless re-encoding), gathers per-core outputs, and applies the greedy NMS
# ordering on the device-computed keys/boxes to emit rois.
#
# Precision: objectness scores are produced as exact-integer hi words (grid
# 2^-22) plus fp32 residuals => |score - exact| ~1e-9, so the descending key
# order reproduces the fp32 reference ordering wherever it is determined.
import numpy as np

FEAT_W = 52
IMG_W = 210.0
SUB = 4
NA = 9
N_CORES = 8
R = 7                  # feature rows per core (8*7 = 56 >= 52, tail zero-padded)
NPOS = R * FEAT_W      # 364 positions per core
PRE_NMS = 12000
POST_NMS = 2000

f32 = np.float32
MAGIC = float(3 * 2 ** 22)

# device channel order (54 rows), all 32-alignment-friendly groups:
#  0:9 obj | 9:18 cls-even | 18:27 l0 | 27:36 l1 | 36:45 l2 | 45:54 l3
def _chan_perm():
    reg = [a * 4 + d for d in range(4) for a in range(NA)]
    cls_even = [36 + 2 * a for a in range(NA)]
    cls_odd = [36 + 2 * a + 1 for a in range(NA)]
    return np.array(cls_odd + cls_even + reg, np.int64)

CHAN_PERM = _chan_perm()

def _magic_round(x, scale=1.0):
    m = f32(MAGIC)
    t = (x.astype(f32) * f32(scale) + m).astype(f32)
    return (t - m).astype(f32)

def _slice3(w, g1, g2, g3):
    a = _magic_round(w, 2.0 ** g1)
    b = _magic_round(w * f32(2.0 ** g2) - a * f32(2.0 ** (g2 - g1)), 1.0)
    c = _magic_round(w * f32(2.0 ** g3) - a * f32(2.0 ** (g3 - g1)) - b * f32(2.0 ** (g3 - g2)), 1.0)
    return a, b, c

def _make_anchor_geom():
    vals = (np.arange(SUB, (FEAT_W + 1) * SUB, SUB) - SUB // 2).astype(np.float64)
    ratios = np.array([0.5, 1.0, 2.0])
    scales = np.array([4.0, 8.0, 16.0])
    h = (SUB * scales[None, :] * np.sqrt(ratios[:, None])).reshape(-1)
    w = (SUB * scales[None, :] * np.sqrt(1.0 / ratios[:, None])).reshape(-1)
    return vals, h.astype(f32), w.astype(f32)

def _exp_poly():
    t = np.linspace(-0.37, 0.37, 4001)
    return np.polyfit(t, np.exp(t), 6).astype(np.float64)

EXP_CO = _exp_poly()

_BUILD_CACHE = {}

def _build():
    if "nc" in _BUILD_CACHE:
        return _BUILD_CACHE["nc"]
    import concourse.bacc as bacc
    import concourse.mybir as mybir
    from concourse.tile import TileContext

    dt = mybir.dt
    op = mybir.AluOpType
    nc = bacc.Bacc("TRN2", target_bir_lowering=False, debug=False, num_devices=N_CORES)

    x_in = nc.declare_dram_parameter("x_in", [128, 2 * 9 * 54], dt.float32, isOutput=False)
    w1a_in = nc.declare_dram_parameter("w1a", [128, 2 * 2304], dt.float16, isOutput=False)
    w1b_in = nc.declare_dram_parameter("w1b", [128, 2 * 2304], dt.float16, isOutput=False)
    w1c_in = nc.declare_dram_parameter("w1c", [128, 2 * 2304], dt.float16, isOutput=False)
    w2abc_in = nc.declare_dram_parameter("w2abc", [128, 2 * 162], dt.float16, isOutput=False)
    w2f_in = nc.declare_dram_parameter("w2f", [128, 2 * 54], dt.float32, isOutput=False)
    b1_in = nc.declare_dram_parameter("b1", [128, 2], dt.float32, isOutput=False)
    b2_in = nc.declare_dram_parameter("b2", [54, 1], dt.float32, isOutput=False)
    acy_in = nc.declare_dram_parameter("acy", [9, NPOS], dt.float32, isOutput=False)
    acx_in = nc.declare_dram_parameter("acx", [9, NPOS], dt.float32, isOutput=False)
    ah_in = nc.declare_dram_parameter("ah", [9, 1], dt.float32, isOutput=False)
    aw_in = nc.declare_dram_parameter("aw", [9, 1], dt.float32, isOutput=False)

    o_scores = nc.declare_dram_parameter("o_scores", [54, NPOS], dt.float32, isOutput=True)
    o_keys = nc.declare_dram_parameter("o_keys", [9, NPOS], dt.int32, isOutput=True)
    o_boxes = nc.declare_dram_parameter("o_boxes", [36, NPOS], dt.float32, isOutput=True)

    with TileContext(nc) as tc:
        with (
            tc.tile_pool(name="sb", bufs=1) as sb,
            tc.tile_pool(name="ps", bufs=2, space="PSUM") as ps,
        ):
            tx = sb.tile([128, 972], dt.float32, tag="tx")
            nc.sync.dma_start(out=tx[:], in_=x_in[:])
            tw2 = sb.tile([128, 324], dt.float16, tag="tw2")
            nc.sync.dma_start(out=tw2[:], in_=w2abc_in[:])
            # weights DMA'd round-major so stage-1 round 0 starts after 1/3 lands
            tw1a = sb.tile([128, 4608], dt.float16, tag="tw1a")
            tw1b = sb.tile([128, 4608], dt.float16, tag="tw1b")
            tw1c = sb.tile([128, 4608], dt.float16, tag="tw1c")
            for rnd in range(3):
                for t_, p_ in ((tw1a, w1a_in), (tw1b, w1b_in), (tw1c, w1c_in)):
                    for cc in range(2):
                        s0 = cc * 2304 + rnd * 768
                        nc.sync.dma_start(out=t_[:, s0:s0 + 768], in_=p_[:, s0:s0 + 768])
            tw2f = sb.tile([128, 108], dt.float32, tag="tw2f")
            nc.sync.dma_start(out=tw2f[:], in_=w2f_in[:])
            tb1 = sb.tile([128, 2], dt.float32, tag="tb1")
            nc.sync.dma_start(out=tb1[:], in_=b1_in[:])
            tb2 = sb.tile([54, 1], dt.float32, tag="tb2")
            nc.sync.dma_start(out=tb2[:], in_=b2_in[:])
            tacy = sb.tile([9, NPOS], dt.float32, tag="tacy")
            tacx = sb.tile([9, NPOS], dt.float32, tag="tacx")
            tah = sb.tile([9, 1], dt.float32, tag="tah")
            taw = sb.tile([9, 1], dt.float32, tag="taw")
            nc.sync.dma_start(out=tacy[:], in_=acy_in[:])
            nc.sync.dma_start(out=tacx[:], in_=acx_in[:])
            nc.sync.dma_start(out=tah[:], in_=ah_in[:])
            nc.sync.dma_start(out=taw[:], in_=aw_in[:])

            # ---- x fixed-point slices ----
            t8 = sb.tile([128, 972], dt.float32, tag="t8")
            nc.vector.tensor_scalar(t8[:], tx[:], float(2.0 ** 8), MAGIC, op.mult, op.add)
            nc.vector.tensor_scalar(t8[:], t8[:], MAGIC, None, op.subtract)
            xa16 = sb.tile([128, 972], dt.float16, tag="xa16")
            nc.vector.tensor_copy(xa16[:], t8[:])
            u = sb.tile([128, 972], dt.float32, tag="u")
            v = sb.tile([128, 972], dt.float32, tag="vv")
            nc.vector.tensor_scalar(v[:], tx[:], float(2.0 ** 19), None, op.mult)
            nc.vector.scalar_tensor_tensor(u[:], t8[:], float(-(2.0 ** 11)), v[:], op.mult, op.add)
            t19 = sb.tile([128, 972], dt.float32, tag="t19")
            nc.vector.tensor_scalar(t19[:], u[:], MAGIC, MAGIC, op.add, op.subtract)
            xb16 = sb.tile([128, 972], dt.float16, tag="xb16")
            nc.vector.tensor_copy(xb16[:], t19[:])
            nc.vector.tensor_scalar(v[:], tx[:], float(2.0 ** 30), None, op.mult)
            nc.vector.scalar_tensor_tensor(u[:], t8[:], float(-(2.0 ** 22)), v[:], op.mult, op.add)
            nc.vector.scalar_tensor_tensor(u[:], t19[:], float(-(2.0 ** 11)), u[:], op.mult, op.add)
            nc.vector.tensor_scalar(u[:], u[:], MAGIC, MAGIC, op.add, op.subtract)
            xc16 = sb.tile([128, 972], dt.float16, tag="xc16")
            nc.vector.tensor_copy(xc16[:], u[:])


            # ---- stage 1: W_eff slices ----
            # L1: per chunk 128 cols: [Wa(0:54) | pad | Wra(64:118) | pad]
            # L1b: per chunk 9 cols: Wrb (obj)
            # L2: per chunk 80 cols: [Wa(0:54) | pad | Wra-obj(64:73) | pad]
            # L3: per chunk 9 cols: Wa-obj
            L1 = sb.tile([128, 18 * 128], dt.float16, tag="L1")
            L1b = sb.tile([128, 18 * 9], dt.float16, tag="L1b")
            L2 = sb.tile([128, 18 * 80], dt.float16, tag="L2")
            L3 = sb.tile([128, 18 * 9], dt.float16, tag="L3")
            nc.vector.memset(L1[:], 0)
            nc.vector.memset(L2[:], 0)
            L1v = L1[:].rearrange("p (c w) -> p c w", w=128)
            L2v = L2[:].rearrange("p (c w) -> p c w", w=80)
            L1bv = L1b[:].rearrange("p (c w) -> p c w", w=9)
            L3v = L3[:].rearrange("p (c w) -> p c w", w=9)
            ROUNDS = (4, 4, 4, 3, 3)
            j0 = 0
            for rnd, nch in enumerate(ROUNDS):
                pstile = ps.tile([128, 2048], dt.float32, tag="arena")
                psv = pstile[:].rearrange("p (c w) -> p c w", w=512)
                for jj in range(nch):
                    j = j0 + jj
                    base = jj * 512
                    for cc in range(2):
                        lhsA = tw1a[:, cc * 2304 + j * 128:cc * 2304 + (j + 1) * 128]
                        lhsB = tw1b[:, cc * 2304 + j * 128:cc * 2304 + (j + 1) * 128]
                        lhsC = tw1c[:, cc * 2304 + j * 128:cc * 2304 + (j + 1) * 128]
                        nc.tensor.matmul(pstile[:, base:base + 162], lhsA,
                                         tw2[:, cc * 162:cc * 162 + 162],
                                         start=(cc == 0), stop=False)
                        nc.tensor.matmul(pstile[:, base + 54:base + 162], lhsB,
                                         tw2[:, cc * 162:cc * 162 + 108],
                                         start=False, stop=False)
                        nc.tensor.matmul(pstile[:, base + 108:base + 162], lhsC,
                                         tw2[:, cc * 162:cc * 162 + 54],
                                         start=False, stop=(cc == 1))
                sl = slice(j0, j0 + nch)
                j0 += nch
                hv = psv[:, 0:nch, :]
                T0 = sb.tile([128, nch, 54], dt.float32, tag="T0")
                nc.vector.tensor_scalar(T0[:], hv[:, :, 0:54], float(2.0 ** -28), None, op.mult)
                WaF = sb.tile([128, nch, 54], dt.float32, tag="WaF")
                nc.vector.tensor_scalar(WaF[:], T0[:], float(2.0 ** 14), MAGIC, op.mult, op.add)
                nc.vector.tensor_scalar(WaF[:], WaF[:], MAGIC, None, op.subtract)
                nc.vector.tensor_copy(L1v[:, sl, 0:54], WaF[:])
                nc.scalar.copy(L2v[:, sl, 0:54], WaF[:])
                nc.scalar.copy(L3v[:, sl, 0:9], WaF[:, :, 0:9])
                Wr1 = sb.tile([128, nch, 54], dt.float32, tag="Wr1")
                nc.vector.scalar_tensor_tensor(Wr1[:], WaF[:], float(-(2.0 ** -14)), T0[:], op.mult, op.add)
                lo = sb.tile([128, nch, 54], dt.float32, tag="lo")
                nc.vector.tensor_scalar(lo[:], hv[:, :, 54:108], float(2.0 ** -40), None, op.mult)
                nc.vector.scalar_tensor_tensor(lo[:], hv[:, :, 108:162], float(2.0 ** -52), lo[:], op.mult, op.add)
                nc.vector.tensor_tensor(out=lo[:], in0=lo[:], in1=Wr1[:], op=op.add)
                WraF = sb.tile([128, nch, 54], dt.float32, tag="WraF")
                nc.vector.tensor_scalar(WraF[:], lo[:], float(2.0 ** 25), MAGIC, op.mult, op.add)
                nc.vector.tensor_scalar(WraF[:], WraF[:], MAGIC, None, op.subtract)
                nc.vector.tensor_copy(L1v[:, sl, 64:118], WraF[:])
                nc.scalar.copy(L2v[:, sl, 64:73], WraF[:, :, 0:9])
                Wrb = sb.tile([128, nch, 9], dt.float32, tag="Wrb")
                nc.vector.scalar_tensor_tensor(Wrb[:], WraF[:, :, 0:9], float(-(2.0 ** -25)),
                                               lo[:, :, 0:9], op.mult, op.add)
                nc.vector.tensor_scalar(Wrb[:], Wrb[:], float(2.0 ** 36), MAGIC, op.mult, op.add)
                nc.vector.tensor_scalar(Wrb[:], Wrb[:], MAGIC, None, op.subtract)
                nc.vector.tensor_copy(L1bv[:, sl, 0:9], Wrb[:])

            # ---- bias offs = w2^T b1 + b2 ----
            psOt = ps.tile([128, 2048], dt.float32, tag="arena")
            psO = psOt[0:54, 0:1]
            for cc in range(2):
                nc.tensor.matmul(psO, tw2f[:, cc * 54:(cc + 1) * 54], tb1[:, cc:cc + 1],
                                 start=(cc == 0), stop=(cc == 1))
            offs = sb.tile([54, 1], dt.float32, tag="offs")
            nc.vector.tensor_tensor(out=offs[:], in0=psO, in1=tb2[:], op=op.add)

            # ---- stage 2: 4 streams into 4 banks ----
            pstile = ps.tile([128, 2048], dt.float32, tag="arena")
            psA = pstile[0:118, 0:NPOS]          # xa x [Wa|pad|Wra]
            psAb = pstile[0:9, 512:512 + NPOS]   # xa x Wrb (obj)
            psB = pstile[0:73, 1024:1024 + NPOS]  # xb x [Wa|pad|Wra-obj]
            psC = pstile[0:9, 1536:1536 + NPOS]  # xc x Wa-obj
            xav = xa16[:].rearrange("p (c r q) -> p c r q", c=2, r=9)
            xbv = xb16[:].rearrange("p (c r q) -> p c r q", c=2, r=9)
            xcv = xc16[:].rearrange("p (c r q) -> p c r q", c=2, r=9)
            for tap in range(9):
                dy, dx = tap // 3, tap % 3
                for cc in range(2):
                    j = tap * 2 + cc
                    st = (j == 0)
                    sp = (j == 17)
                    ra = xav[:, cc, dy:dy + 7, dx:dx + 52]
                    rb = xbv[:, cc, dy:dy + 7, dx:dx + 52]
                    rc = xcv[:, cc, dy:dy + 7, dx:dx + 52]
                    nc.tensor.matmul(psA, L1[:, j * 128:j * 128 + 118], ra, start=st, stop=sp)
                    nc.tensor.matmul(psAb, L1b[:, j * 9:(j + 1) * 9], ra, start=st, stop=sp)
                    nc.tensor.matmul(psB, L2[:, j * 80:j * 80 + 73], rb, start=st, stop=sp)
                    nc.tensor.matmul(psC, L3[:, j * 9:(j + 1) * 9], rc, start=st, stop=sp)

            # ---- score combine ----
            c1 = sb.tile([54, NPOS], dt.float32, tag="c1")
            nc.vector.tensor_scalar(c1[:], pstile[64:118, 0:NPOS], float(2.0 ** -33), None, op.mult)
            Sres = sb.tile([54, NPOS], dt.float32, tag="Sres")
            nc.vector.scalar_tensor_tensor(Sres[:], pstile[0:54, 1024:1024 + NPOS],
                                           float(2.0 ** -33), c1[:], op.mult, op.add)
            e1 = sb.tile([9, NPOS], dt.float32, tag="e1")
            nc.vector.tensor_scalar(e1[:], psAb, float(2.0 ** -44), None, op.mult)
            nc.vector.scalar_tensor_tensor(e1[:], pstile[64:73, 1024:1024 + NPOS],
                                           float(2.0 ** -44), e1[:], op.mult, op.add)
            nc.vector.scalar_tensor_tensor(e1[:], psC, float(2.0 ** -44), e1[:], op.mult, op.add)
            nc.vector.tensor_tensor(out=Sres[0:9, :], in0=Sres[0:9, :], in1=e1[:], op=op.add)
            nc.vector.tensor_scalar(Sres[:], Sres[:], offs[:], None, op.add)
            S54 = sb.tile([54, NPOS], dt.float32, tag="S54")
            nc.vector.scalar_tensor_tensor(S54[:], pstile[0:54, 0:NPOS], float(2.0 ** -22),
                                           Sres[:], op.mult, op.add)
            nc.sync.dma_start(out=o_scores[:], in_=S54[:])

            # ---- int32 keys (before masking) ----
            kr = sb.tile([9, NPOS], dt.float32, tag="kr")
            nc.vector.tensor_scalar(kr[:], Sres[0:9, :], float(2.0 ** 31), MAGIC, op.mult, op.add)
            nc.vector.tensor_scalar(kr[:], kr[:], MAGIC, None, op.subtract)
            kri = sb.tile([9, NPOS], dt.int32, tag="kri")
            nc.vector.tensor_copy(kri[:], kr[:])
            k1 = sb.tile([9, NPOS], dt.int32, tag="k1")
            nc.vector.tensor_copy(k1[:], pstile[0:9, 0:NPOS])
            key = sb.tile([9, NPOS], dt.int32, tag="key")
            nc.vector.tensor_scalar(key[:], k1[:], 512, None, op.mult)
            nc.vector.tensor_tensor(out=key[:], in0=key[:], in1=kri[:], op=op.add)

            # ---- decode boxes ----
            Sl = []
            for d in range(4):
                t = sb.tile([9, NPOS], dt.float32, tag=f"Sl{d}")
                nc.sync.dma_start(out=t[:], in_=S54[18 + d * 9:27 + d * 9, :])
                Sl.append(t)
            cy = sb.tile([9, NPOS], dt.float32, tag="cy")
            cx = sb.tile([9, NPOS], dt.float32, tag="cx")
            nc.vector.scalar_tensor_tensor(cy[:], Sl[0][:], tah[:], tacy[:], op.mult, op.add)
            nc.vector.scalar_tensor_tensor(cx[:], Sl[1][:], taw[:], tacx[:], op.mult, op.add)
            eh = sb.tile([9, NPOS], dt.float32, tag="eh")
            ew = sb.tile([9, NPOS], dt.float32, tag="ew")
            for (srct, dstt) in ((Sl[2], eh), (Sl[3], ew)):
                nc.vector.tensor_scalar(dstt[:], srct[:], float(EXP_CO[0]), float(EXP_CO[1]),
                                        op.mult, op.add)
                for ci in range(2, 7):
                    nc.vector.tensor_tensor(out=dstt[:], in0=dstt[:], in1=srct[:], op=op.mult)
                    nc.vector.tensor_scalar(dstt[:], dstt[:], float(EXP_CO[ci]), None, op.add)
            nc.vector.tensor_scalar(eh[:], eh[:], tah[:], 0.5, op.mult, op.mult)
            nc.vector.tensor_scalar(ew[:], ew[:], taw[:], 0.5, op.mult, op.mult)
            bco = []
            for (ctr, half, sgn) in ((cy, eh, op.subtract), (cx, ew, op.subtract),
                                     (cy, eh, op.add), (cx, ew, op.add)):
                t = sb.tile([9, NPOS], dt.float32, tag=f"bco{len(bco)}")
                nc.vector.tensor_tensor(out=t[:], in0=ctr[:], in1=half[:], op=sgn)
                nc.vector.tensor_scalar(t[:], t[:], 0.0, float(IMG_W), op.max, op.min)
                nc.sync.dma_start(out=o_boxes[len(bco) * 9:(len(bco) + 1) * 9, :], in_=t[:])
                bco.append(t)
            hs = sb.tile([9, NPOS], dt.float32, tag="hs")
            ws = sb.tile([9, NPOS], dt.float32, tag="ws")
            nc.vector.tensor_tensor(out=hs[:], in0=bco[2][:], in1=bco[0][:], op=op.subtract)
            nc.vector.tensor_tensor(out=ws[:], in0=bco[3][:], in1=bco[1][:], op=op.subtract)
            nc.vector.tensor_scalar(hs[:], hs[:], 16.0, None, op.is_ge)
            nc.vector.tensor_scalar(ws[:], ws[:], 16.0, None, op.is_ge)
            nc.vector.tensor_tensor(out=hs[:], in0=hs[:], in1=ws[:], op=op.mult)
            vi = sb.tile([9, NPOS], dt.int32, tag="vi")
            nc.vector.tensor_copy(vi[:], hs[:])
            nc.vector.tensor_scalar(key[:], key[:], 2 ** 30, None, op.add)
            nc.vector.tensor_tensor(out=key[:], in0=key[:], in1=vi[:], op=op.mult)
            nc.vector.tensor_scalar(key[:], key[:], 2 ** 30, None, op.subtract)
            nc.sync.dma_start(out=o_keys[:], in_=key[:])

    nc.compile()
    _BUILD_CACHE["nc"] = nc
    return nc


def _host_inputs(x, conv1_w, conv1_b, reg_w, reg_b, cls_w, cls_b):
    x = np.asarray(x, f32)[0]
    w1 = np.asarray(conv1_w, f32)
    b1 = np.asarray(conv1_b, f32)
    wr = np.asarray(reg_w, f32)[:, :, 0, 0]
    br = np.asarray(reg_b, f32)
    wc = np.asarray(cls_w, f32)[:, :, 0, 0]
    bc = np.asarray(cls_b, f32)

    w2_all = np.concatenate([wr, wc], 0)
    b2_all = np.concatenate([br, bc], 0)
    w2p = np.ascontiguousarray(w2_all[CHAN_PERM].T.astype(f32))
    b2p = b2_all[CHAN_PERM].astype(f32)

    w1p = np.ascontiguousarray(w1.transpose(0, 2, 3, 1).reshape(256, 2304).astype(f32))
    w1a, w1b, w1c = _slice3(w1p, 14, 26, 38)
    w2a, w2b, w2c = _slice3(w2p, 14, 26, 38)

    def two_chunk(m):
        F = m.shape[1]
        out = np.empty((128, 2 * F), m.dtype)
        out[:, :F] = m[:128]
        out[:, F:] = m[128:]
        return out

    w1a_h = two_chunk(w1a).astype(np.float16)
    w1b_h = two_chunk(w1b).astype(np.float16)
    w1c_h = two_chunk(w1c).astype(np.float16)
    w2abc_h = two_chunk(np.concatenate([w2a, w2b, w2c], 1)).astype(np.float16)
    w2f_h = two_chunk(w2p).astype(f32)
    b1_h = np.ascontiguousarray(b1.reshape(2, 128).T.astype(f32))
    b2_h = b2p.reshape(54, 1)

    vals, ahv, awv = _make_anchor_geom()
    ah_h = ahv.reshape(9, 1).astype(f32)
    aw_h = awv.reshape(9, 1).astype(f32)

    xpad = np.zeros((256, 58, 54), f32)
    xpad[:, 1:53, 1:53] = x
    valsf = vals.astype(f32)
    in_maps = []
    for c in range(N_CORES):
        r0 = c * R
        stripe = xpad[:, r0:r0 + 9, :]
        x_h = two_chunk(np.ascontiguousarray(stripe.reshape(256, 9 * 54)))
        acy = np.zeros((9, NPOS), f32)
        acx = np.zeros((9, NPOS), f32)
        for rr in range(R):
            gr = r0 + rr
            acy[:, rr * 52:(rr + 1) * 52] = valsf[gr] if gr < FEAT_W else f32(0)
            acx[:, rr * 52:(rr + 1) * 52] = valsf[None, :FEAT_W]
        in_maps.append({
            "x_in": x_h, "w1a": w1a_h, "w1b": w1b_h, "w1c": w1c_h,
            "w2abc": w2abc_h, "w2f": w2f_h, "b1": b1_h, "b2": b2_h,
            "acy": acy, "acx": acx, "ah": ah_h, "aw": aw_h,
        })
    return in_maps


def _host_nms(keys, boxes):
    order = np.argsort(-keys.astype(np.int64), kind="stable")[:PRE_NMS]
    b = boxes[order]
    areas = (b[:, 3] - b[:, 1] + f32(1)) * (b[:, 2] - b[:, 0] + f32(1))
    N = len(order)
    idx = np.arange(N)
    supp = np.zeros(N, bool)
    kept = []
    for i in range(N):
        if supp[i]:
            continue
        kept.append(i)
        if len(kept) == POST_NMS:
            break
        bi = b[i]
        yy1 = np.maximum(bi[0], b[:, 0])
        xx1 = np.maximum(bi[1], b[:, 1])
        yy2 = np.minimum(bi[2], b[:, 2])
        xx2 = np.minimum(bi[3], b[:, 3])
        inter = np.maximum(f32(0), xx2 - xx1 + f32(1)) * np.maximum(f32(0), yy2 - yy1 + f32(1))
        iou = inter / (areas[i] + areas - inter)
        supp |= (iou > f32(0.7)) & (idx > i)
    out = np.zeros((POST_NMS, 4), f32)
    k = np.array(kept[:POST_NMS])
    out[:len(k)] = b[k]
    return out


TRACE = False
LAST_RESULT = [None]


def kernel(x, conv1_w, conv1_b, reg_w, reg_b, cls_w, cls_b):
    from concourse.bass_utils import run_bass_kernel_spmd
    nc = _build()
    in_maps = _host_inputs(x, conv1_w, conv1_b, reg_w, reg_b, cls_w, cls_b)
    res = run_bass_kernel_spmd(nc, in_maps, list(range(N_CORES)), trace=TRACE)
    LAST_RESULT[0] = res

    scores = np.zeros((54, 56 * 52), f32)
    keys = np.zeros((9, 56 * 52), np.int32)
    boxes = np.zeros((36, 56 * 52), f32)
    for c in range(N_CORES):
        r = res.results[c]
        scores[:, c * NPOS:(c + 1) * NPOS] = r["o_scores"]
        keys[:, c * NPOS:(c + 1) * NPOS] = np.asarray(r["o_keys"]).view(np.int32)
        boxes[:, c * NPOS:(c + 1) * NPOS] = r["o_boxes"]
    npos = FEAT_W * FEAT_W
    scores = scores[:, :npos]
    keys = keys[:, :npos]
    boxes = boxes[:, :npos]

    locs = np.empty((npos, NA, 4), f32)
    for d in range(4):
        locs[:, :, d] = scores[18 + d * 9:27 + d * 9, :].T
    locs = locs.reshape(1, npos * NA, 4)
    cls_scores = np.empty((npos, NA, 2), f32)
    cls_scores[:, :, 0] = scores[9:18, :].T
    cls_scores[:, :, 1] = scores[0:9, :].T
    cls_scores = cls_scores.reshape(1, npos * NA, 2)
    obj = np.ascontiguousarray(cls_scores[:, :, 1].reshape(1, npos * NA))

    kflat = keys.T.reshape(-1)
    bflat = np.stack([boxes[0:9].T.reshape(-1), boxes[9:18].T.reshape(-1),
                      boxes[18:27].T.reshape(-1), boxes[27:36].T.reshape(-1)], 1)
    rois = _host_nms(kflat, bflat)
    return (rois, locs, cls_scores, obj, cls_scores)


# revision 16
# speedup vs baseline: 1.1148x; 1.0475x over previous
# Trainium2 Bass kernel for nn_RPNmodel (RPN conv head + proposal decode + NMS).
#
# Device (8 NeuronCores, SPMD, feature rows sharded 7 rows/core):
#   - effective-weight contraction W_eff[k,o] = sum_c w1[c,k]*w2[o,c] on the PE,
#     using fp16 fixed-point slices with exact integer accumulation in PSUM
#   - conv-as-GEMM over the 9 taps: scores + locs for the core's positions (PE)
#   - box decode (polynomial exp), validity, int32 sort keys (DVE)
# Host: shards/pads inputs, slices weights into fixed-point fp16 words (a
# lossless re-encoding), gathers per-core outputs, and applies the greedy NMS
# ordering on the device-computed keys/boxes to emit rois.
#
# Precision: objectness scores are produced as exact-integer hi words (grid
# 2^-22) plus fp32 residuals => |score - exact| ~1e-9, so the descending key
# order reproduces the fp32 reference ordering wherever it is determined.
import numpy as np

FEAT_W = 52
IMG_W = 210.0
SUB = 4
NA = 9
N_CORES = 8
R = 7                  # feature rows per core (8*7 = 56 >= 52, tail zero-padded)
NPOS = R * FEAT_W      # 364 positions per core
PRE_NMS = 12000
POST_NMS = 2000

f32 = np.float32
MAGIC = float(3 * 2 ** 22)

# device channel order (54 rows), all 32-alignment-friendly groups:
#  0:9 obj | 9:18 cls-even | 18:27 l0 | 27:36 l1 | 36:45 l2 | 45:54 l3
def _chan_perm():
    reg = [a * 4 + d for d in range(4) for a in range(NA)]
    cls_even = [36 + 2 * a for a in range(NA)]
    cls_odd = [36 + 2 * a + 1 for a in range(NA)]
    return np.array(cls_odd + cls_even + reg, np.int64)

CHAN_PERM = _chan_perm()

def _magic_round(x, scale=1.0):
    m = f32(MAGIC)
    t = (x.astype(f32) * f32(scale) + m).astype(f32)
    return (t - m).astype(f32)

def _slice3(w, g1, g2, g3):
    a = _magic_round(w, 2.0 ** g1)
    b = _magic_round(w * f32(2.0 ** g2) - a * f32(2.0 ** (g2 - g1)), 1.0)
    c = _magic_round(w * f32(2.0 ** g3) - a * f32(2.0 ** (g3 - g1)) - b * f32(2.0 ** (g3 - g2)), 1.0)
    return a, b, c

def _make_anchor_geom():
    vals = (np.arange(SUB, (FEAT_W + 1) * SUB, SUB) - SUB // 2).astype(np.float64)
    ratios = np.array([0.5, 1.0, 2.0])
    scales = np.array([4.0, 8.0, 16.0])
    h = (SUB * scales[None, :] * np.sqrt(ratios[:, None])).reshape(-1)
    w = (SUB * scales[None, :] * np.sqrt(1.0 / ratios[:, None])).reshape(-1)
    return vals, h.astype(f32), w.astype(f32)

def _exp_poly():
    t = np.linspace(-0.37, 0.37, 4001)
    return np.polyfit(t, np.exp(t), 6).astype(np.float64)

EXP_CO = _exp_poly()

_BUILD_CACHE = {}

def _build():
    if "nc" in _BUILD_CACHE:
        return _BUILD_CACHE["nc"]
    import concourse.bacc as bacc
    import concourse.mybir as mybir
    from concourse.tile import TileContext

    dt = mybir.dt
    op = mybir.AluOpType
    nc = bacc.Bacc("TRN2", target_bir_lowering=False, debug=False, num_devices=N_CORES)

    x_in = nc.declare_dram_parameter("x_in", [128, 2 * 9 * 54], dt.float32, isOutput=False)
    w1a_in = nc.declare_dram_parameter("w1a", [128, 2 * 2304], dt.float16, isOutput=False)
    w1b_in = nc.declare_dram_parameter("w1b", [128, 2 * 2304], dt.float16, isOutput=False)
    w1c_in = nc.declare_dram_parameter("w1c", [128, 2 * 2304], dt.float16, isOutput=False)
    w2abc_in = nc.declare_dram_parameter("w2abc", [128, 2 * 162], dt.float16, isOutput=False)
    w2f_in = nc.declare_dram_parameter("w2f", [128, 2 * 54], dt.float32, isOutput=False)
    b1_in = nc.declare_dram_parameter("b1", [128, 2], dt.float32, isOutput=False)
    b2_in = nc.declare_dram_parameter("b2", [54, 1], dt.float32, isOutput=False)
    acy_in = nc.declare_dram_parameter("acy", [9, NPOS], dt.float32, isOutput=False)
    acx_in = nc.declare_dram_parameter("acx", [9, NPOS], dt.float32, isOutput=False)
    ah_in = nc.declare_dram_parameter("ah", [9, 1], dt.float32, isOutput=False)
    aw_in = nc.declare_dram_parameter("aw", [9, 1], dt.float32, isOutput=False)

    o_scores = nc.declare_dram_parameter("o_scores", [54, NPOS], dt.float32, isOutput=True)
    o_keys = nc.declare_dram_parameter("o_keys", [9, NPOS], dt.int32, isOutput=True)
    o_boxes = nc.declare_dram_parameter("o_boxes", [36, NPOS], dt.float32, isOutput=True)

    with TileContext(nc) as tc:
        with (
            tc.tile_pool(name="sb", bufs=1) as sb,
            tc.tile_pool(name="ps", bufs=2, space="PSUM") as ps,
        ):
            tx = sb.tile([128, 972], dt.float32, tag="tx")
            nc.sync.dma_start(out=tx[:], in_=x_in[:])
            tw2 = sb.tile([128, 324], dt.float16, tag="tw2")
            nc.sync.dma_start(out=tw2[:], in_=w2abc_in[:])
            # weights DMA'd round-major so stage-1 round 0 starts after 1/3 lands
            tw1a = sb.tile([128, 4608], dt.float16, tag="tw1a")
            tw1b = sb.tile([128, 4608], dt.float16, tag="tw1b")
            tw1c = sb.tile([128, 4608], dt.float16, tag="tw1c")
            for rnd in range(3):
                for t_, p_ in ((tw1a, w1a_in), (tw1b, w1b_in), (tw1c, w1c_in)):
                    for cc in range(2):
                        s0 = cc * 2304 + rnd * 768
                        nc.sync.dma_start(out=t_[:, s0:s0 + 768], in_=p_[:, s0:s0 + 768])
            tw2f = sb.tile([128, 108], dt.float32, tag="tw2f")
            nc.sync.dma_start(out=tw2f[:], in_=w2f_in[:])
            tb1 = sb.tile([128, 2], dt.float32, tag="tb1")
            nc.sync.dma_start(out=tb1[:], in_=b1_in[:])
            tb2 = sb.tile([54, 1], dt.float32, tag="tb2")
            nc.sync.dma_start(out=tb2[:], in_=b2_in[:])
            tacy = sb.tile([9, NPOS], dt.float32, tag="tacy")
            tacx = sb.tile([9, NPOS], dt.float32, tag="tacx")
            tah = sb.tile([9, 1], dt.float32, tag="tah")
            taw = sb.tile([9, 1], dt.float32, tag="taw")
            nc.sync.dma_start(out=tacy[:], in_=acy_in[:])
            nc.sync.dma_start(out=tacx[:], in_=acx_in[:])
            nc.sync.dma_start(out=tah[:], in_=ah_in[:])
            nc.sync.dma_start(out=taw[:], in_=aw_in[:])

            # ---- x fixed-point slices ----
            t8 = sb.tile([128, 972], dt.float32, tag="t8")
            nc.vector.tensor_scalar(t8[:], tx[:], float(2.0 ** 8), MAGIC, op.mult, op.add)
            nc.vector.tensor_scalar(t8[:], t8[:], MAGIC, None, op.subtract)
            xa16 = sb.tile([128, 972], dt.float16, tag="xa16")
            nc.vector.tensor_copy(xa16[:], t8[:])
            u = sb.tile([128, 972], dt.float32, tag="u")
            v = sb.tile([128, 972], dt.float32, tag="vv")
            nc.vector.tensor_scalar(v[:], tx[:], float(2.0 ** 19), None, op.mult)
            nc.vector.scalar_tensor_tensor(u[:], t8[:], float(-(2.0 ** 11)), v[:], op.mult, op.add)
            t19 = sb.tile([128, 972], dt.float32, tag="t19")
            nc.vector.tensor_scalar(t19[:], u[:], MAGIC, MAGIC, op.add, op.subtract)
            xb16 = sb.tile([128, 972], dt.float16, tag="xb16")
            nc.vector.tensor_copy(xb16[:], t19[:])
            nc.vector.tensor_scalar(v[:], tx[:], float(2.0 ** 30), None, op.mult)
            nc.vector.scalar_tensor_tensor(u[:], t8[:], float(-(2.0 ** 22)), v[:], op.mult, op.add)
            nc.vector.scalar_tensor_tensor(u[:], t19[:], float(-(2.0 ** 11)), u[:], op.mult, op.add)
            nc.vector.tensor_scalar(u[:], u[:], MAGIC, MAGIC, op.add, op.subtract)
            xc16 = sb.tile([128, 972], dt.float16, tag="xc16")
            nc.vector.tensor_copy(xc16[:], u[:])


            # ---- stage 1: W_eff slices ----
            # L1: per chunk 128 cols: [Wa(0:54) | pad | Wra(64:118) | pad]
            # L1b: per chunk 9 cols: Wrb (obj)
            # L2: per chunk 80 cols: [Wa(0:54) | pad | Wra-obj(64:73) | pad]
            # L3: per chunk 9 cols: Wa-obj
            L1 = sb.tile([128, 18 * 128], dt.float16, tag="L1")
            L1b = sb.tile([128, 18 * 9], dt.float16, tag="L1b")
            L2 = sb.tile([128, 18 * 80], dt.float16, tag="L2")
            L3 = sb.tile([128, 18 * 9], dt.float16, tag="L3")
            nc.vector.memset(L1[:], 0)
            nc.vector.memset(L2[:], 0)
            L1v = L1[:].rearrange("p (c w) -> p c w", w=128)
            L2v = L2[:].rearrange("p (c w) -> p c w", w=80)
            L1bv = L1b[:].rearrange("p (c w) -> p c w", w=9)
            L3v = L3[:].rearrange("p (c w) -> p c w", w=9)
            ROUNDS = (4, 4, 4, 3, 3)
            j0 = 0
            for rnd, nch in enumerate(ROUNDS):
                pstile = ps.tile([128, 2048], dt.float32, tag="arena")
                psv = pstile[:].rearrange("p (c w) -> p c w", w=512)
                for jj in range(nch):
                    j = j0 + jj
                    base = jj * 512
                    for cc in range(2):
                        lhsA = tw1a[:, cc * 2304 + j * 128:cc * 2304 + (j + 1) * 128]
                        lhsB = tw1b[:, cc * 2304 + j * 128:cc * 2304 + (j + 1) * 128]
                        lhsC = tw1c[:, cc * 2304 + j * 128:cc * 2304 + (j + 1) * 128]
                        nc.tensor.matmul(pstile[:, base:base + 162], lhsA,
                                         tw2[:, cc * 162:cc * 162 + 162],
                                         start=(cc == 0), stop=False)
                        nc.tensor.matmul(pstile[:, base + 54:base + 162], lhsB,
                                         tw2[:, cc * 162:cc * 162 + 108],
                                         start=False, stop=False)
                        nc.tensor.matmul(pstile[:, base + 108:base + 162], lhsC,
                                         tw2[:, cc * 162:cc * 162 + 54],
                                         start=False, stop=(cc == 1))
                sl = slice(j0, j0 + nch)
                j0 += nch
                hv = psv[:, 0:nch, :]
                T0 = sb.tile([128, nch, 54], dt.float32, tag="T0")
                nc.scalar.mul(T0[:], hv[:, :, 0:54], float(2.0 ** -28))
                WaF = sb.tile([128, nch, 54], dt.float32, tag="WaF")
                nc.vector.tensor_scalar(WaF[:], T0[:], float(2.0 ** 14), MAGIC, op.mult, op.add)
                nc.vector.tensor_scalar(WaF[:], WaF[:], MAGIC, None, op.subtract)
                nc.vector.tensor_copy(L1v[:, sl, 0:54], WaF[:])
                nc.scalar.copy(L2v[:, sl, 0:54], WaF[:])
                nc.scalar.copy(L3v[:, sl, 0:9], WaF[:, :, 0:9])
                Wr1 = sb.tile([128, nch, 54], dt.float32, tag="Wr1")
                nc.vector.scalar_tensor_tensor(Wr1[:], WaF[:], float(-(2.0 ** -14)), T0[:], op.mult, op.add)
                lo = sb.tile([128, nch, 54], dt.float32, tag="lo")
                nc.scalar.mul(lo[:], hv[:, :, 54:108], float(2.0 ** -40))
                nc.vector.scalar_tensor_tensor(lo[:], hv[:, :, 108:162], float(2.0 ** -52), lo[:], op.mult, op.add)
                nc.vector.tensor_tensor(out=lo[:], in0=lo[:], in1=Wr1[:], op=op.add)
                WraF = sb.tile([128, nch, 54], dt.float32, tag="WraF")
                nc.vector.tensor_scalar(WraF[:], lo[:], float(2.0 ** 25), MAGIC, op.mult, op.add)
                nc.vector.tensor_scalar(WraF[:], WraF[:], MAGIC, None, op.subtract)
                nc.vector.tensor_copy(L1v[:, sl, 64:118], WraF[:])
                nc.scalar.copy(L2v[:, sl, 64:73], WraF[:, :, 0:9])
                Wrb = sb.tile([128, nch, 9], dt.float32, tag="Wrb")
                nc.vector.scalar_tensor_tensor(Wrb[:], WraF[:, :, 0:9], float(-(2.0 ** -25)),
                                               lo[:, :, 0:9], op.mult, op.add)
                nc.vector.tensor_scalar(Wrb[:], Wrb[:], float(2.0 ** 36), MAGIC, op.mult, op.add)
                nc.vector.tensor_scalar(Wrb[:], Wrb[:], MAGIC, None, op.subtract)
                nc.vector.tensor_copy(L1bv[:, sl, 0:9], Wrb[:])

            # ---- bias offs = w2^T b1 + b2 ----
            psOt = ps.tile([128, 2048], dt.float32, tag="arena")
            psO = psOt[0:54, 0:1]
            for cc in range(2):
                nc.tensor.matmul(psO, tw2f[:, cc * 54:(cc + 1) * 54], tb1[:, cc:cc + 1],
                                 start=(cc == 0), stop=(cc == 1))
            offs = sb.tile([54, 1], dt.float32, tag="offs")
            nc.vector.tensor_tensor(out=offs[:], in0=psO, in1=tb2[:], op=op.add)

            # ---- stage 2: 4 streams into 4 banks ----
            pstile = ps.tile([128, 2048], dt.float32, tag="arena")
            psA = pstile[0:118, 0:NPOS]          # xa x [Wa|pad|Wra]
            psAb = pstile[0:9, 512:512 + NPOS]   # xa x Wrb (obj)
            psB = pstile[0:73, 1024:1024 + NPOS]  # xb x [Wa|pad|Wra-obj]
            psC = pstile[0:9, 1536:1536 + NPOS]  # xc x Wa-obj
            xav = xa16[:].rearrange("p (c r q) -> p c r q", c=2, r=9)
            xbv = xb16[:].rearrange("p (c r q) -> p c r q", c=2, r=9)
            xcv = xc16[:].rearrange("p (c r q) -> p c r q", c=2, r=9)
            for tap in range(9):
                dy, dx = tap // 3, tap % 3
                for cc in range(2):
                    j = tap * 2 + cc
                    st = (j == 0)
                    sp = (j == 17)
                    ra = xav[:, cc, dy:dy + 7, dx:dx + 52]
                    rb = xbv[:, cc, dy:dy + 7, dx:dx + 52]
                    rc = xcv[:, cc, dy:dy + 7, dx:dx + 52]
                    nc.tensor.matmul(psA, L1[:, j * 128:j * 128 + 118], ra, start=st, stop=sp)
                    nc.tensor.matmul(psAb, L1b[:, j * 9:(j + 1) * 9], ra, start=st, stop=sp)
                    nc.tensor.matmul(psB, L2[:, j * 80:j * 80 + 73], rb, start=st, stop=sp)
                    nc.tensor.matmul(psC, L3[:, j * 9:(j + 1) * 9], rc, start=st, stop=sp)

            # ---- score combine ----
            c1 = sb.tile([54, NPOS], dt.float32, tag="c1")
            nc.scalar.mul(c1[:], pstile[64:118, 0:NPOS], float(2.0 ** -33))
            Sres = sb.tile([54, NPOS], dt.float32, tag="Sres")
            nc.vector.scalar_tensor_tensor(Sres[:], pstile[0:54, 1024:1024 + NPOS],
                                           float(2.0 ** -33), c1[:], op.mult, op.add)
            e1 = sb.tile([9, NPOS], dt.float32, tag="e1")
            nc.scalar.mul(e1[:], psAb, float(2.0 ** -44))
            nc.vector.scalar_tensor_tensor(e1[:], pstile[64:73, 1024:1024 + NPOS],
                                           float(2.0 ** -44), e1[:], op.mult, op.add)
            nc.vector.scalar_tensor_tensor(e1[:], psC, float(2.0 ** -44), e1[:], op.mult, op.add)
            nc.vector.tensor_tensor(out=Sres[0:9, :], in0=Sres[0:9, :], in1=e1[:], op=op.add)
            nc.vector.tensor_scalar(Sres[:], Sres[:], offs[:], None, op.add)
            S54 = sb.tile([54, NPOS], dt.float32, tag="S54")
            nc.vector.scalar_tensor_tensor(S54[:], pstile[0:54, 0:NPOS], float(2.0 ** -22),
                                           Sres[:], op.mult, op.add)
            nc.sync.dma_start(out=o_scores[:], in_=S54[:])

            # ---- int32 keys (before masking) ----
            kr = sb.tile([9, NPOS], dt.float32, tag="kr")
            nc.vector.tensor_scalar(kr[:], Sres[0:9, :], float(2.0 ** 31), MAGIC, op.mult, op.add)
            nc.vector.tensor_scalar(kr[:], kr[:], MAGIC, None, op.subtract)
            kri = sb.tile([9, NPOS], dt.int32, tag="kri")
            nc.vector.tensor_copy(kri[:], kr[:])
            k1 = sb.tile([9, NPOS], dt.int32, tag="k1")
            nc.vector.tensor_copy(k1[:], pstile[0:9, 0:NPOS])
            key = sb.tile([9, NPOS], dt.int32, tag="key")
            nc.vector.tensor_scalar(key[:], k1[:], 512, None, op.mult)
            nc.vector.tensor_tensor(out=key[:], in0=key[:], in1=kri[:], op=op.add)

            # ---- decode boxes ----
            Sl = []
            for d in range(4):
                t = sb.tile([9, NPOS], dt.float32, tag=f"Sl{d}")
                nc.sync.dma_start(out=t[:], in_=S54[18 + d * 9:27 + d * 9, :])
                Sl.append(t)
            cy = sb.tile([9, NPOS], dt.float32, tag="cy")
            cx = sb.tile([9, NPOS], dt.float32, tag="cx")
            nc.vector.scalar_tensor_tensor(cy[:], Sl[0][:], tah[:], tacy[:], op.mult, op.add)
            nc.vector.scalar_tensor_tensor(cx[:], Sl[1][:], taw[:], tacx[:], op.mult, op.add)
            eh = sb.tile([9, NPOS], dt.float32, tag="eh")
            ew = sb.tile([9, NPOS], dt.float32, tag="ew")
            for (srct, dstt) in ((Sl[2], eh), (Sl[3], ew)):
                nc.vector.tensor_scalar(dstt[:], srct[:], float(EXP_CO[0]), float(EXP_CO[1]),
                                        op.mult, op.add)
                for ci in range(2, 7):
                    nc.vector.tensor_tensor(out=dstt[:], in0=dstt[:], in1=srct[:], op=op.mult)
                    nc.vector.tensor_scalar(dstt[:], dstt[:], float(EXP_CO[ci]), None, op.add)
            nc.vector.tensor_scalar(eh[:], eh[:], tah[:], 0.5, op.mult, op.mult)
            nc.vector.tensor_scalar(ew[:], ew[:], taw[:], 0.5, op.mult, op.mult)
            bco = []
            for (ctr, half, sgn) in ((cy, eh, op.subtract), (cx, ew, op.subtract),
                                     (cy, eh, op.add), (cx, ew, op.add)):
                t = sb.tile([9, NPOS], dt.float32, tag=f"bco{len(bco)}")
                nc.vector.tensor_tensor(out=t[:], in0=ctr[:], in1=half[:], op=sgn)
                nc.vector.tensor_scalar(t[:], t[:], 0.0, float(IMG_W), op.max, op.min)
                nc.sync.dma_start(out=o_boxes[len(bco) * 9:(len(bco) + 1) * 9, :], in_=t[:])
                bco.append(t)
            hs = sb.tile([9, NPOS], dt.float32, tag="hs")
            ws = sb.tile([9, NPOS], dt.float32, tag="ws")
            nc.vector.tensor_tensor(out=hs[:], in0=bco[2][:], in1=bco[0][:], op=op.subtract)
            nc.vector.tensor_tensor(out=ws[:], in0=bco[3][:], in1=bco[1][:], op=op.subtract)
            nc.vector.tensor_scalar(hs[:], hs[:], 16.0, None, op.is_ge)
            nc.vector.tensor_scalar(ws[:], ws[:], 16.0, None, op.is_ge)
            nc.vector.tensor_tensor(out=hs[:], in0=hs[:], in1=ws[:], op=op.mult)
            vi = sb.tile([9, NPOS], dt.int32, tag="vi")
            nc.vector.tensor_copy(vi[:], hs[:])
            nc.vector.tensor_scalar(key[:], key[:], 2 ** 30, None, op.add)
            nc.vector.tensor_tensor(out=key[:], in0=key[:], in1=vi[:], op=op.mult)
            nc.vector.tensor_scalar(key[:], key[:], 2 ** 30, None, op.subtract)
            nc.sync.dma_start(out=o_keys[:], in_=key[:])

    nc.compile()
    _BUILD_CACHE["nc"] = nc
    return nc


def _host_inputs(x, conv1_w, conv1_b, reg_w, reg_b, cls_w, cls_b):
    x = np.asarray(x, f32)[0]
    w1 = np.asarray(conv1_w, f32)
    b1 = np.asarray(conv1_b, f32)
    wr = np.asarray(reg_w, f32)[:, :, 0, 0]
    br = np.asarray(reg_b, f32)
    wc = np.asarray(cls_w, f32)[:, :, 0, 0]
    bc = np.asarray(cls_b, f32)

    w2_all = np.concatenate([wr, wc], 0)
    b2_all = np.concatenate([br, bc], 0)
    w2p = np.ascontiguousarray(w2_all[CHAN_PERM].T.astype(f32))
    b2p = b2_all[CHAN_PERM].astype(f32)

    w1p = np.ascontiguousarray(w1.transpose(0, 2, 3, 1).reshape(256, 2304).astype(f32))
    w1a, w1b, w1c = _slice3(w1p, 14, 26, 38)
    w2a, w2b, w2c = _slice3(w2p, 14, 26, 38)

    def two_chunk(m):
        F = m.shape[1]
        out = np.empty((128, 2 * F), m.dtype)
        out[:, :F] = m[:128]
        out[:, F:] = m[128:]
        return out

    w1a_h = two_chunk(w1a).astype(np.float16)
    w1b_h = two_chunk(w1b).astype(np.float16)
    w1c_h = two_chunk(w1c).astype(np.float16)
    w2abc_h = two_chunk(np.concatenate([w2a, w2b, w2c], 1)).astype(np.float16)
    w2f_h = two_chunk(w2p).astype(f32)
    b1_h = np.ascontiguousarray(b1.reshape(2, 128).T.astype(f32))
    b2_h = b2p.reshape(54, 1)

    vals, ahv, awv = _make_anchor_geom()
    ah_h = ahv.reshape(9, 1).astype(f32)
    aw_h = awv.reshape(9, 1).astype(f32)

    xpad = np.zeros((256, 58, 54), f32)
    xpad[:, 1:53, 1:53] = x
    valsf = vals.astype(f32)
    in_maps = []
    for c in range(N_CORES):
        r0 = c * R
        stripe = xpad[:, r0:r0 + 9, :]
        x_h = two_chunk(np.ascontiguousarray(stripe.reshape(256, 9 * 54)))
        acy = np.zeros((9, NPOS), f32)
        acx = np.zeros((9, NPOS), f32)
        for rr in range(R):
            gr = r0 + rr
            acy[:, rr * 52:(rr + 1) * 52] = valsf[gr] if gr < FEAT_W else f32(0)
            acx[:, rr * 52:(rr + 1) * 52] = valsf[None, :FEAT_W]
        in_maps.append({
            "x_in": x_h, "w1a": w1a_h, "w1b": w1b_h, "w1c": w1c_h,
            "w2abc": w2abc_h, "w2f": w2f_h, "b1": b1_h, "b2": b2_h,
            "acy": acy, "acx": acx, "ah": ah_h, "aw": aw_h,
        })
    return in_maps


def _host_nms(keys, boxes):
    order = np.argsort(-keys.astype(np.int64), kind="stable")[:PRE_NMS]
    b = boxes[order]
    areas = (b[:, 3] - b[:, 1] + f32(1)) * (b[:, 2] - b[:, 0] + f32(1))
    N = len(order)
    idx = np.arange(N)
    supp = np.zeros(N, bool)
    kept = []
    for i in range(N):
        if supp[i]:
            continue
        kept.append(i)
        if len(kept) == POST_NMS:
            break
        bi = b[i]
        yy1 = np.maximum(bi[0], b[:, 0])
        xx1 = np.maximum(bi[1], b[:, 1])
        yy2 = np.minimum(bi[2], b[:, 2])
        xx2 = np.minimum(bi[3], b[:, 3])
        inter = np.maximum(f32(0), xx2 - xx1 + f32(1)) * np.maximum(f32(0), yy2 - yy1 + f32(1))
        iou = inter / (areas[i] + areas - inter)
        supp |= (iou > f32(0.7)) & (idx > i)
    out = np.zeros((POST_NMS, 4), f32)
    k = np.array(kept[:POST_NMS])
    out[:len(k)] = b[k]
    return out


TRACE = False
LAST_RESULT = [None]


def kernel(x, conv1_w, conv1_b, reg_w, reg_b, cls_w, cls_b):
    from concourse.bass_utils import run_bass_kernel_spmd
    nc = _build()
    in_maps = _host_inputs(x, conv1_w, conv1_b, reg_w, reg_b, cls_w, cls_b)
    res = run_bass_kernel_spmd(nc, in_maps, list(range(N_CORES)), trace=TRACE)
    LAST_RESULT[0] = res

    scores = np.zeros((54, 56 * 52), f32)
    keys = np.zeros((9, 56 * 52), np.int32)
    boxes = np.zeros((36, 56 * 52), f32)
    for c in range(N_CORES):
        r = res.results[c]
        scores[:, c * NPOS:(c + 1) * NPOS] = r["o_scores"]
        keys[:, c * NPOS:(c + 1) * NPOS] = np.asarray(r["o_keys"]).view(np.int32)
        boxes[:, c * NPOS:(c + 1) * NPOS] = r["o_boxes"]
    npos = FEAT_W * FEAT_W
    scores = scores[:, :npos]
    keys = keys[:, :npos]
    boxes = boxes[:, :npos]

    locs = np.empty((npos, NA, 4), f32)
    for d in range(4):
        locs[:, :, d] = scores[18 + d * 9:27 + d * 9, :].T
    locs = locs.reshape(1, npos * NA, 4)
    cls_scores = np.empty((npos, NA, 2), f32)
    cls_scores[:, :, 0] = scores[9:18, :].T
    cls_scores[:, :, 1] = scores[0:9, :].T
    cls_scores = cls_scores.reshape(1, npos * NA, 2)
    obj = np.ascontiguousarray(cls_scores[:, :, 1].reshape(1, npos * NA))

    kflat = keys.T.reshape(-1)
    bflat = np.stack([boxes[0:9].T.reshape(-1), boxes[9:18].T.reshape(-1),
                      boxes[18:27].T.reshape(-1), boxes[27:36].T.reshape(-1)], 1)
    rois = _host_nms(kflat, bflat)
    return (rois, locs, cls_scores, obj, cls_scores)


# revision 17
# speedup vs baseline: 1.1407x; 1.0232x over previous
# Trainium2 Bass kernel for nn_RPNmodel (RPN conv head + proposal decode + NMS).
#
# Device (8 NeuronCores, SPMD, feature rows sharded 7 rows/core):
#   - effective-weight contraction W_eff[k,o] = sum_c w1[c,k]*w2[o,c] on the PE,
#     using fp16 fixed-point slices with exact integer accumulation in PSUM
#   - conv-as-GEMM over the 9 taps: scores + locs for the core's positions (PE)
#   - box decode (polynomial exp), validity, int32 sort keys (DVE)
# Host: shards/pads inputs, slices weights into fixed-point fp16 words (a
# lossless re-encoding), gathers per-core outputs, and applies the greedy NMS
# ordering on the device-computed keys/boxes to emit rois.
#
# Precision: objectness scores are produced as exact-integer hi words (grid
# 2^-22) plus fp32 residuals => |score - exact| ~1e-9, so the descending key
# order reproduces the fp32 reference ordering wherever it is determined.
import numpy as np

FEAT_W = 52
IMG_W = 210.0
SUB = 4
NA = 9
N_CORES = 8
R = 7                  # feature rows per core (8*7 = 56 >= 52, tail zero-padded)
NPOS = R * FEAT_W      # 364 positions per core
PRE_NMS = 12000
POST_NMS = 2000

f32 = np.float32
MAGIC = float(3 * 2 ** 22)

# device channel order (54 rows), all 32-alignment-friendly groups:
#  0:9 obj | 9:18 cls-even | 18:27 l0 | 27:36 l1 | 36:45 l2 | 45:54 l3
def _chan_perm():
    reg = [a * 4 + d for d in range(4) for a in range(NA)]
    cls_even = [36 + 2 * a for a in range(NA)]
    cls_odd = [36 + 2 * a + 1 for a in range(NA)]
    return np.array(cls_odd + cls_even + reg, np.int64)

CHAN_PERM = _chan_perm()

def _magic_round(x, scale=1.0):
    m = f32(MAGIC)
    t = (x.astype(f32) * f32(scale) + m).astype(f32)
    return (t - m).astype(f32)

def _slice3(w, g1, g2, g3):
    a = _magic_round(w, 2.0 ** g1)
    b = _magic_round(w * f32(2.0 ** g2) - a * f32(2.0 ** (g2 - g1)), 1.0)
    c = _magic_round(w * f32(2.0 ** g3) - a * f32(2.0 ** (g3 - g1)) - b * f32(2.0 ** (g3 - g2)), 1.0)
    return a, b, c

def _make_anchor_geom():
    vals = (np.arange(SUB, (FEAT_W + 1) * SUB, SUB) - SUB // 2).astype(np.float64)
    ratios = np.array([0.5, 1.0, 2.0])
    scales = np.array([4.0, 8.0, 16.0])
    h = (SUB * scales[None, :] * np.sqrt(ratios[:, None])).reshape(-1)
    w = (SUB * scales[None, :] * np.sqrt(1.0 / ratios[:, None])).reshape(-1)
    return vals, h.astype(f32), w.astype(f32)

def _exp_poly():
    t = np.linspace(-0.37, 0.37, 4001)
    return np.polyfit(t, np.exp(t), 5).astype(np.float64)

EXP_CO = _exp_poly()

_BUILD_CACHE = {}

def _build():
    if "nc" in _BUILD_CACHE:
        return _BUILD_CACHE["nc"]
    import concourse.bacc as bacc
    import concourse.mybir as mybir
    from concourse.tile import TileContext

    dt = mybir.dt
    op = mybir.AluOpType
    nc = bacc.Bacc("TRN2", target_bir_lowering=False, debug=False, num_devices=N_CORES)

    x_in = nc.declare_dram_parameter("x_in", [128, 2 * 9 * 54], dt.float32, isOutput=False)
    w1a_in = nc.declare_dram_parameter("w1a", [128, 2 * 2304], dt.float16, isOutput=False)
    w1b_in = nc.declare_dram_parameter("w1b", [128, 2 * 2304], dt.float16, isOutput=False)
    w1c_in = nc.declare_dram_parameter("w1c", [128, 2 * 2304], dt.float16, isOutput=False)
    w2abc_in = nc.declare_dram_parameter("w2abc", [128, 2 * 162], dt.float16, isOutput=False)
    w2f_in = nc.declare_dram_parameter("w2f", [128, 2 * 54], dt.float32, isOutput=False)
    b1_in = nc.declare_dram_parameter("b1", [128, 2], dt.float32, isOutput=False)
    b2_in = nc.declare_dram_parameter("b2", [54, 1], dt.float32, isOutput=False)
    acy_in = nc.declare_dram_parameter("acy", [9, NPOS], dt.float32, isOutput=False)
    acx_in = nc.declare_dram_parameter("acx", [9, NPOS], dt.float32, isOutput=False)
    ah_in = nc.declare_dram_parameter("ah", [9, 1], dt.float32, isOutput=False)
    aw_in = nc.declare_dram_parameter("aw", [9, 1], dt.float32, isOutput=False)

    o_scores = nc.declare_dram_parameter("o_scores", [54, NPOS], dt.float32, isOutput=True)
    o_keys = nc.declare_dram_parameter("o_keys", [9, NPOS], dt.int32, isOutput=True)
    o_boxes = nc.declare_dram_parameter("o_boxes", [36, NPOS], dt.float32, isOutput=True)

    with TileContext(nc) as tc:
        with (
            tc.tile_pool(name="sb", bufs=1) as sb,
            tc.tile_pool(name="ps", bufs=2, space="PSUM") as ps,
        ):
            tx = sb.tile([128, 972], dt.float32, tag="tx")
            nc.sync.dma_start(out=tx[:], in_=x_in[:])
            tw2 = sb.tile([128, 324], dt.float16, tag="tw2")
            nc.sync.dma_start(out=tw2[:], in_=w2abc_in[:])
            # weights DMA'd round-major so stage-1 round 0 starts after 1/3 lands
            tw1a = sb.tile([128, 4608], dt.float16, tag="tw1a")
            tw1b = sb.tile([128, 4608], dt.float16, tag="tw1b")
            tw1c = sb.tile([128, 4608], dt.float16, tag="tw1c")
            for rnd in range(3):
                for t_, p_ in ((tw1a, w1a_in), (tw1b, w1b_in), (tw1c, w1c_in)):
                    for cc in range(2):
                        s0 = cc * 2304 + rnd * 768
                        nc.sync.dma_start(out=t_[:, s0:s0 + 768], in_=p_[:, s0:s0 + 768])
            tw2f = sb.tile([128, 108], dt.float32, tag="tw2f")
            nc.sync.dma_start(out=tw2f[:], in_=w2f_in[:])
            tb1 = sb.tile([128, 2], dt.float32, tag="tb1")
            nc.sync.dma_start(out=tb1[:], in_=b1_in[:])
            tb2 = sb.tile([54, 1], dt.float32, tag="tb2")
            nc.sync.dma_start(out=tb2[:], in_=b2_in[:])
            tacy = sb.tile([9, NPOS], dt.float32, tag="tacy")
            tacx = sb.tile([9, NPOS], dt.float32, tag="tacx")
            tah = sb.tile([9, 1], dt.float32, tag="tah")
            taw = sb.tile([9, 1], dt.float32, tag="taw")
            nc.sync.dma_start(out=tacy[:], in_=acy_in[:])
            nc.sync.dma_start(out=tacx[:], in_=acx_in[:])
            nc.sync.dma_start(out=tah[:], in_=ah_in[:])
            nc.sync.dma_start(out=taw[:], in_=aw_in[:])

            # ---- x fixed-point slices ----
            t8 = sb.tile([128, 972], dt.float32, tag="t8")
            nc.vector.tensor_scalar(t8[:], tx[:], float(2.0 ** 8), MAGIC, op.mult, op.add)
            nc.vector.tensor_scalar(t8[:], t8[:], MAGIC, None, op.subtract)
            xa16 = sb.tile([128, 972], dt.float16, tag="xa16")
            nc.vector.tensor_copy(xa16[:], t8[:])
            u = sb.tile([128, 972], dt.float32, tag="u")
            v = sb.tile([128, 972], dt.float32, tag="vv")
            nc.vector.tensor_scalar(v[:], tx[:], float(2.0 ** 19), None, op.mult)
            nc.vector.scalar_tensor_tensor(u[:], t8[:], float(-(2.0 ** 11)), v[:], op.mult, op.add)
            t19 = sb.tile([128, 972], dt.float32, tag="t19")
            nc.vector.tensor_scalar(t19[:], u[:], MAGIC, MAGIC, op.add, op.subtract)
            xb16 = sb.tile([128, 972], dt.float16, tag="xb16")
            nc.vector.tensor_copy(xb16[:], t19[:])
            nc.vector.tensor_scalar(v[:], tx[:], float(2.0 ** 30), None, op.mult)
            nc.vector.scalar_tensor_tensor(u[:], t8[:], float(-(2.0 ** 22)), v[:], op.mult, op.add)
            nc.vector.scalar_tensor_tensor(u[:], t19[:], float(-(2.0 ** 11)), u[:], op.mult, op.add)
            nc.vector.tensor_scalar(u[:], u[:], MAGIC, MAGIC, op.add, op.subtract)
            xc16 = sb.tile([128, 972], dt.float16, tag="xc16")
            nc.vector.tensor_copy(xc16[:], u[:])


            # ---- stage 1: W_eff slices ----
            # L1: per chunk 128 cols: [Wa(0:54) | pad | Wra(64:118) | pad]
            # L1b: per chunk 9 cols: Wrb (obj)
            # L2: per chunk 80 cols: [Wa(0:54) | pad | Wra-obj(64:73) | pad]
            # L3: per chunk 9 cols: Wa-obj
            L1 = sb.tile([128, 18 * 128], dt.float16, tag="L1")
            L1b = sb.tile([128, 18 * 9], dt.float16, tag="L1b")
            L2 = sb.tile([128, 18 * 80], dt.float16, tag="L2")
            L3 = sb.tile([128, 18 * 9], dt.float16, tag="L3")
            nc.vector.memset(L1[:], 0)
            nc.vector.memset(L2[:], 0)
            L1v = L1[:].rearrange("p (c w) -> p c w", w=128)
            L2v = L2[:].rearrange("p (c w) -> p c w", w=80)
            L1bv = L1b[:].rearrange("p (c w) -> p c w", w=9)
            L3v = L3[:].rearrange("p (c w) -> p c w", w=9)
            ROUNDS = (4, 4, 4, 3, 3)
            j0 = 0
            for rnd, nch in enumerate(ROUNDS):
                pstile = ps.tile([128, 2048], dt.float32, tag="arena")
                psv = pstile[:].rearrange("p (c w) -> p c w", w=512)
                for jj in range(nch):
                    j = j0 + jj
                    base = jj * 512
                    for cc in range(2):
                        lhsA = tw1a[:, cc * 2304 + j * 128:cc * 2304 + (j + 1) * 128]
                        lhsB = tw1b[:, cc * 2304 + j * 128:cc * 2304 + (j + 1) * 128]
                        lhsC = tw1c[:, cc * 2304 + j * 128:cc * 2304 + (j + 1) * 128]
                        nc.tensor.matmul(pstile[:, base:base + 162], lhsA,
                                         tw2[:, cc * 162:cc * 162 + 162],
                                         start=(cc == 0), stop=False)
                        nc.tensor.matmul(pstile[:, base + 54:base + 162], lhsB,
                                         tw2[:, cc * 162:cc * 162 + 108],
                                         start=False, stop=False)
                        nc.tensor.matmul(pstile[:, base + 108:base + 162], lhsC,
                                         tw2[:, cc * 162:cc * 162 + 54],
                                         start=False, stop=(cc == 1))
                sl = slice(j0, j0 + nch)
                j0 += nch
                hv = psv[:, 0:nch, :]
                T0 = sb.tile([128, nch, 54], dt.float32, tag="T0")
                nc.scalar.mul(T0[:], hv[:, :, 0:54], float(2.0 ** -28))
                WaF = sb.tile([128, nch, 54], dt.float32, tag="WaF")
                nc.vector.tensor_scalar(WaF[:], T0[:], float(2.0 ** 14), MAGIC, op.mult, op.add)
                nc.vector.tensor_scalar(WaF[:], WaF[:], MAGIC, None, op.subtract)
                nc.vector.tensor_copy(L1v[:, sl, 0:54], WaF[:])
                nc.scalar.copy(L2v[:, sl, 0:54], WaF[:])
                nc.scalar.copy(L3v[:, sl, 0:9], WaF[:, :, 0:9])
                Wr1 = sb.tile([128, nch, 54], dt.float32, tag="Wr1")
                nc.vector.scalar_tensor_tensor(Wr1[:], WaF[:], float(-(2.0 ** -14)), T0[:], op.mult, op.add)
                lo = sb.tile([128, nch, 54], dt.float32, tag="lo")
                nc.scalar.mul(lo[:], hv[:, :, 54:108], float(2.0 ** -40))
                nc.vector.scalar_tensor_tensor(lo[:], hv[:, :, 108:162], float(2.0 ** -52), lo[:], op.mult, op.add)
                nc.vector.tensor_tensor(out=lo[:], in0=lo[:], in1=Wr1[:], op=op.add)
                WraF = sb.tile([128, nch, 54], dt.float32, tag="WraF")
                nc.vector.tensor_scalar(WraF[:], lo[:], float(2.0 ** 25), MAGIC, op.mult, op.add)
                nc.vector.tensor_scalar(WraF[:], WraF[:], MAGIC, None, op.subtract)
                nc.vector.tensor_copy(L1v[:, sl, 64:118], WraF[:])
                nc.scalar.copy(L2v[:, sl, 64:73], WraF[:, :, 0:9])
                Wrb = sb.tile([128, nch, 9], dt.float32, tag="Wrb")
                nc.vector.scalar_tensor_tensor(Wrb[:], WraF[:, :, 0:9], float(-(2.0 ** -25)),
                                               lo[:, :, 0:9], op.mult, op.add)
                nc.vector.tensor_scalar(Wrb[:], Wrb[:], float(2.0 ** 36), MAGIC, op.mult, op.add)
                nc.vector.tensor_scalar(Wrb[:], Wrb[:], MAGIC, None, op.subtract)
                nc.vector.tensor_copy(L1bv[:, sl, 0:9], Wrb[:])

            # ---- bias offs = w2^T b1 + b2 ----
            psOt = ps.tile([128, 2048], dt.float32, tag="arena")
            psO = psOt[0:54, 0:1]
            for cc in range(2):
                nc.tensor.matmul(psO, tw2f[:, cc * 54:(cc + 1) * 54], tb1[:, cc:cc + 1],
                                 start=(cc == 0), stop=(cc == 1))
            offs = sb.tile([54, 1], dt.float32, tag="offs")
            nc.vector.tensor_tensor(out=offs[:], in0=psO, in1=tb2[:], op=op.add)

            # ---- stage 2: 4 streams into 4 banks ----
            pstile = ps.tile([128, 2048], dt.float32, tag="arena")
            psA = pstile[0:118, 0:NPOS]          # xa x [Wa|pad|Wra]
            psAb = pstile[0:9, 512:512 + NPOS]   # xa x Wrb (obj)
            psB = pstile[0:73, 1024:1024 + NPOS]  # xb x [Wa|pad|Wra-obj]
            psC = pstile[0:9, 1536:1536 + NPOS]  # xc x Wa-obj
            xav = xa16[:].rearrange("p (c r q) -> p c r q", c=2, r=9)
            xbv = xb16[:].rearrange("p (c r q) -> p c r q", c=2, r=9)
            xcv = xc16[:].rearrange("p (c r q) -> p c r q", c=2, r=9)
            for tap in range(9):
                dy, dx = tap // 3, tap % 3
                for cc in range(2):
                    j = tap * 2 + cc
                    st = (j == 0)
                    sp = (j == 17)
                    ra = xav[:, cc, dy:dy + 7, dx:dx + 52]
                    rb = xbv[:, cc, dy:dy + 7, dx:dx + 52]
                    rc = xcv[:, cc, dy:dy + 7, dx:dx + 52]
                    nc.tensor.matmul(psA, L1[:, j * 128:j * 128 + 118], ra, start=st, stop=sp)
                    nc.tensor.matmul(psAb, L1b[:, j * 9:(j + 1) * 9], ra, start=st, stop=sp)
                    nc.tensor.matmul(psB, L2[:, j * 80:j * 80 + 73], rb, start=st, stop=sp)
                    nc.tensor.matmul(psC, L3[:, j * 9:(j + 1) * 9], rc, start=st, stop=sp)

            # ---- score combine ----
            c1 = sb.tile([54, NPOS], dt.float32, tag="c1")
            nc.scalar.mul(c1[:], pstile[64:118, 0:NPOS], float(2.0 ** -33))
            Sres = sb.tile([54, NPOS], dt.float32, tag="Sres")
            nc.vector.scalar_tensor_tensor(Sres[:], pstile[0:54, 1024:1024 + NPOS],
                                           float(2.0 ** -33), c1[:], op.mult, op.add)
            e1 = sb.tile([9, NPOS], dt.float32, tag="e1")
            nc.scalar.mul(e1[:], psAb, float(2.0 ** -44))
            nc.vector.scalar_tensor_tensor(e1[:], pstile[64:73, 1024:1024 + NPOS],
                                           float(2.0 ** -44), e1[:], op.mult, op.add)
            nc.vector.scalar_tensor_tensor(e1[:], psC, float(2.0 ** -44), e1[:], op.mult, op.add)
            nc.vector.tensor_tensor(out=Sres[0:9, :], in0=Sres[0:9, :], in1=e1[:], op=op.add)
            nc.vector.tensor_scalar(Sres[:], Sres[:], offs[:], None, op.add)
            S54 = sb.tile([54, NPOS], dt.float32, tag="S54")
            nc.vector.scalar_tensor_tensor(S54[:], pstile[0:54, 0:NPOS], float(2.0 ** -22),
                                           Sres[:], op.mult, op.add)
            nc.sync.dma_start(out=o_scores[:], in_=S54[:])

            # ---- int32 keys (before masking) ----
            kr = sb.tile([9, NPOS], dt.float32, tag="kr")
            nc.vector.tensor_scalar(kr[:], Sres[0:9, :], float(2.0 ** 31), MAGIC, op.mult, op.add)
            nc.vector.tensor_scalar(kr[:], kr[:], MAGIC, None, op.subtract)
            kri = sb.tile([9, NPOS], dt.int32, tag="kri")
            nc.vector.tensor_copy(kri[:], kr[:])
            k1 = sb.tile([9, NPOS], dt.int32, tag="k1")
            nc.vector.tensor_copy(k1[:], pstile[0:9, 0:NPOS])
            key = sb.tile([9, NPOS], dt.int32, tag="key")
            nc.vector.tensor_scalar(key[:], k1[:], 512, None, op.mult)
            nc.vector.tensor_tensor(out=key[:], in0=key[:], in1=kri[:], op=op.add)

            # ---- decode boxes ----
            Sl = []
            for d in range(4):
                t = sb.tile([9, NPOS], dt.float32, tag=f"Sl{d}")
                nc.sync.dma_start(out=t[:], in_=S54[18 + d * 9:27 + d * 9, :])
                Sl.append(t)
            cy = sb.tile([9, NPOS], dt.float32, tag="cy")
            cx = sb.tile([9, NPOS], dt.float32, tag="cx")
            nc.vector.scalar_tensor_tensor(cy[:], Sl[0][:], tah[:], tacy[:], op.mult, op.add)
            nc.vector.scalar_tensor_tensor(cx[:], Sl[1][:], taw[:], tacx[:], op.mult, op.add)
            eh = sb.tile([9, NPOS], dt.float32, tag="eh")
            ew = sb.tile([9, NPOS], dt.float32, tag="ew")
            for (srct, dstt) in ((Sl[2], eh), (Sl[3], ew)):
                nc.vector.tensor_scalar(dstt[:], srct[:], float(EXP_CO[0]), float(EXP_CO[1]),
                                        op.mult, op.add)
                for ci in range(2, 6):
                    nc.vector.tensor_tensor(out=dstt[:], in0=dstt[:], in1=srct[:], op=op.mult)
                    nc.vector.tensor_scalar(dstt[:], dstt[:], float(EXP_CO[ci]), None, op.add)
            nc.vector.tensor_scalar(eh[:], eh[:], tah[:], 0.5, op.mult, op.mult)
            nc.vector.tensor_scalar(ew[:], ew[:], taw[:], 0.5, op.mult, op.mult)
            bco = []
            for (ctr, half, sgn) in ((cy, eh, op.subtract), (cx, ew, op.subtract),
                                     (cy, eh, op.add), (cx, ew, op.add)):
                t = sb.tile([9, NPOS], dt.float32, tag=f"bco{len(bco)}")
                nc.vector.tensor_tensor(out=t[:], in0=ctr[:], in1=half[:], op=sgn)
                nc.vector.tensor_scalar(t[:], t[:], 0.0, float(IMG_W), op.max, op.min)
                nc.sync.dma_start(out=o_boxes[len(bco) * 9:(len(bco) + 1) * 9, :], in_=t[:])
                bco.append(t)
            hs = sb.tile([9, NPOS], dt.float32, tag="hs")
            ws = sb.tile([9, NPOS], dt.float32, tag="ws")
            nc.vector.tensor_tensor(out=hs[:], in0=bco[2][:], in1=bco[0][:], op=op.subtract)
            nc.vector.tensor_tensor(out=ws[:], in0=bco[3][:], in1=bco[1][:], op=op.subtract)
            nc.vector.tensor_scalar(hs[:], hs[:], 16.0, None, op.is_ge)
            nc.vector.tensor_scalar(ws[:], ws[:], 16.0, None, op.is_ge)
            nc.vector.tensor_tensor(out=hs[:], in0=hs[:], in1=ws[:], op=op.mult)
            vi = sb.tile([9, NPOS], dt.int32, tag="vi")
            nc.vector.tensor_copy(vi[:], hs[:])
            nc.vector.tensor_scalar(key[:], key[:], 2 ** 30, None, op.add)
            nc.vector.tensor_tensor(out=key[:], in0=key[:], in1=vi[:], op=op.mult)
            nc.vector.tensor_scalar(key[:], key[:], 2 ** 30, None, op.subtract)
            nc.sync.dma_start(out=o_keys[:], in_=key[:])

    nc.compile()
    _BUILD_CACHE["nc"] = nc
    return nc


def _host_inputs(x, conv1_w, conv1_b, reg_w, reg_b, cls_w, cls_b):
    x = np.asarray(x, f32)[0]
    w1 = np.asarray(conv1_w, f32)
    b1 = np.asarray(conv1_b, f32)
    wr = np.asarray(reg_w, f32)[:, :, 0, 0]
    br = np.asarray(reg_b, f32)
    wc = np.asarray(cls_w, f32)[:, :, 0, 0]
    bc = np.asarray(cls_b, f32)

    w2_all = np.concatenate([wr, wc], 0)
    b2_all = np.concatenate([br, bc], 0)
    w2p = np.ascontiguousarray(w2_all[CHAN_PERM].T.astype(f32))
    b2p = b2_all[CHAN_PERM].astype(f32)

    w1p = np.ascontiguousarray(w1.transpose(0, 2, 3, 1).reshape(256, 2304).astype(f32))
    w1a, w1b, w1c = _slice3(w1p, 14, 26, 38)
    w2a, w2b, w2c = _slice3(w2p, 14, 26, 38)

    def two_chunk(m):
        F = m.shape[1]
        out = np.empty((128, 2 * F), m.dtype)
        out[:, :F] = m[:128]
        out[:, F:] = m[128:]
        return out

    w1a_h = two_chunk(w1a).astype(np.float16)
    w1b_h = two_chunk(w1b).astype(np.float16)
    w1c_h = two_chunk(w1c).astype(np.float16)
    w2abc_h = two_chunk(np.concatenate([w2a, w2b, w2c], 1)).astype(np.float16)
    w2f_h = two_chunk(w2p).astype(f32)
    b1_h = np.ascontiguousarray(b1.reshape(2, 128).T.astype(f32))
    b2_h = b2p.reshape(54, 1)

    vals, ahv, awv = _make_anchor_geom()
    ah_h = ahv.reshape(9, 1).astype(f32)
    aw_h = awv.reshape(9, 1).astype(f32)

    xpad = np.zeros((256, 58, 54), f32)
    xpad[:, 1:53, 1:53] = x
    valsf = vals.astype(f32)
    in_maps = []
    for c in range(N_CORES):
        r0 = c * R
        stripe = xpad[:, r0:r0 + 9, :]
        x_h = two_chunk(np.ascontiguousarray(stripe.reshape(256, 9 * 54)))
        acy = np.zeros((9, NPOS), f32)
        acx = np.zeros((9, NPOS), f32)
        for rr in range(R):
            gr = r0 + rr
            acy[:, rr * 52:(rr + 1) * 52] = valsf[gr] if gr < FEAT_W else f32(0)
            acx[:, rr * 52:(rr + 1) * 52] = valsf[None, :FEAT_W]
        in_maps.append({
            "x_in": x_h, "w1a": w1a_h, "w1b": w1b_h, "w1c": w1c_h,
            "w2abc": w2abc_h, "w2f": w2f_h, "b1": b1_h, "b2": b2_h,
            "acy": acy, "acx": acx, "ah": ah_h, "aw": aw_h,
        })
    return in_maps


def _host_nms(keys, boxes):
    order = np.argsort(-keys.astype(np.int64), kind="stable")[:PRE_NMS]
    b = boxes[order]
    areas = (b[:, 3] - b[:, 1] + f32(1)) * (b[:, 2] - b[:, 0] + f32(1))
    N = len(order)
    idx = np.arange(N)
    supp = np.zeros(N, bool)
    kept = []
    for i in range(N):
        if supp[i]:
            continue
        kept.append(i)
        if len(kept) == POST_NMS:
            break
        bi = b[i]
        yy1 = np.maximum(bi[0], b[:, 0])
        xx1 = np.maximum(bi[1], b[:, 1])
        yy2 = np.minimum(bi[2], b[:, 2])
        xx2 = np.minimum(bi[3], b[:, 3])
        inter = np.maximum(f32(0), xx2 - xx1 + f32(1)) * np.maximum(f32(0), yy2 - yy1 + f32(1))
        iou = inter / (areas[i] + areas - inter)
        supp |= (iou > f32(0.7)) & (idx > i)
    out = np.zeros((POST_NMS, 4), f32)
    k = np.array(kept[:POST_NMS])
    out[:len(k)] = b[k]
    return out


TRACE = False
LAST_RESULT = [None]


def kernel(x, conv1_w, conv1_b, reg_w, reg_b, cls_w, cls_b):
    from concourse.bass_utils import run_bass_kernel_spmd
    nc = _build()
    in_maps = _host_inputs(x, conv1_w, conv1_b, reg_w, reg_b, cls_w, cls_b)
    res = run_bass_kernel_spmd(nc, in_maps, list(range(N_CORES)), trace=TRACE)
    LAST_RESULT[0] = res

    scores = np.zeros((54, 56 * 52), f32)
    keys = np.zeros((9, 56 * 52), np.int32)
    boxes = np.zeros((36, 56 * 52), f32)
    for c in range(N_CORES):
        r = res.results[c]
        scores[:, c * NPOS:(c + 1) * NPOS] = r["o_scores"]
        keys[:, c * NPOS:(c + 1) * NPOS] = np.asarray(r["o_keys"]).view(np.int32)
        boxes[:, c * NPOS:(c + 1) * NPOS] = r["o_boxes"]
    npos = FEAT_W * FEAT_W
    scores = scores[:, :npos]
    keys = keys[:, :npos]
    boxes = boxes[:, :npos]

    locs = np.empty((npos, NA, 4), f32)
    for d in range(4):
        locs[:, :, d] = scores[18 + d * 9:27 + d * 9, :].T
    locs = locs.reshape(1, npos * NA, 4)
    cls_scores = np.empty((npos, NA, 2), f32)
    cls_scores[:, :, 0] = scores[9:18, :].T
    cls_scores[:, :, 1] = scores[0:9, :].T
    cls_scores = cls_scores.reshape(1, npos * NA, 2)
    obj = np.ascontiguousarray(cls_scores[:, :, 1].reshape(1, npos * NA))

    kflat = keys.T.reshape(-1)
    bflat = np.stack([boxes[0:9].T.reshape(-1), boxes[9:18].T.reshape(-1),
                      boxes[18:27].T.reshape(-1), boxes[27:36].T.reshape(-1)], 1)
    rois = _host_nms(kflat, bflat)
    return (rois, locs, cls_scores, obj, cls_scores)


# revision 18
# speedup vs baseline: 1.1767x; 1.0315x over previous
# Trainium2 Bass kernel for nn_RPNmodel (RPN conv head + proposal decode + NMS).
#
# Device (8 NeuronCores, SPMD, feature rows sharded 7 rows/core):
#   - effective-weight contraction W_eff[k,o] = sum_c w1[c,k]*w2[o,c] on the PE,
#     using fp16 fixed-point slices with exact integer accumulation in PSUM
#   - conv-as-GEMM over the 9 taps: scores + locs for the core's positions (PE)
#   - box decode (polynomial exp), validity, int32 sort keys (DVE)
# Host: shards/pads inputs, slices weights into fixed-point fp16 words (a
# lossless re-encoding), gathers per-core outputs, and applies the greedy NMS
# ordering on the device-computed keys/boxes to emit rois.
#
# Precision: objectness scores are produced as exact-integer hi words (grid
# 2^-22) plus fp32 residuals => |score - exact| ~1e-9, so the descending key
# order reproduces the fp32 reference ordering wherever it is determined.
import numpy as np

FEAT_W = 52
IMG_W = 210.0
SUB = 4
NA = 9
N_CORES = 8
R = 7                  # feature rows per core (8*7 = 56 >= 52, tail zero-padded)
NPOS = R * FEAT_W      # 364 positions per core
PRE_NMS = 12000
POST_NMS = 2000

f32 = np.float32
MAGIC = float(3 * 2 ** 22)

# device channel order (54 rows), all 32-alignment-friendly groups:
#  0:9 obj | 9:18 cls-even | 18:27 l0 | 27:36 l1 | 36:45 l2 | 45:54 l3
def _chan_perm():
    reg = [a * 4 + d for d in range(4) for a in range(NA)]
    cls_even = [36 + 2 * a for a in range(NA)]
    cls_odd = [36 + 2 * a + 1 for a in range(NA)]
    return np.array(cls_odd + cls_even + reg, np.int64)

CHAN_PERM = _chan_perm()

def _magic_round(x, scale=1.0):
    m = f32(MAGIC)
    t = (x.astype(f32) * f32(scale) + m).astype(f32)
    return (t - m).astype(f32)

def _slice3(w, g1, g2, g3):
    a = _magic_round(w, 2.0 ** g1)
    b = _magic_round(w * f32(2.0 ** g2) - a * f32(2.0 ** (g2 - g1)), 1.0)
    c = _magic_round(w * f32(2.0 ** g3) - a * f32(2.0 ** (g3 - g1)) - b * f32(2.0 ** (g3 - g2)), 1.0)
    return a, b, c

def _make_anchor_geom():
    vals = (np.arange(SUB, (FEAT_W + 1) * SUB, SUB) - SUB // 2).astype(np.float64)
    ratios = np.array([0.5, 1.0, 2.0])
    scales = np.array([4.0, 8.0, 16.0])
    h = (SUB * scales[None, :] * np.sqrt(ratios[:, None])).reshape(-1)
    w = (SUB * scales[None, :] * np.sqrt(1.0 / ratios[:, None])).reshape(-1)
    return vals, h.astype(f32), w.astype(f32)

def _exp_poly():
    t = np.linspace(-0.37, 0.37, 4001)
    return np.polyfit(t, np.exp(t), 5).astype(np.float64)

EXP_CO = _exp_poly()

_BUILD_CACHE = {}

def _build():
    if "nc" in _BUILD_CACHE:
        return _BUILD_CACHE["nc"]
    import concourse.bacc as bacc
    import concourse.mybir as mybir
    from concourse.tile import TileContext

    dt = mybir.dt
    op = mybir.AluOpType
    nc = bacc.Bacc("TRN2", target_bir_lowering=False, debug=False, num_devices=N_CORES)

    x_in = nc.declare_dram_parameter("x_in", [128, 2 * 9 * 54], dt.float32, isOutput=False)
    w1a_in = nc.declare_dram_parameter("w1a", [128, 2 * 2304], dt.float16, isOutput=False)
    w1b_in = nc.declare_dram_parameter("w1b", [128, 2 * 2304], dt.float16, isOutput=False)
    w1c_in = nc.declare_dram_parameter("w1c", [128, 2 * 2304], dt.float16, isOutput=False)
    w2abc_in = nc.declare_dram_parameter("w2abc", [128, 2 * 162], dt.float16, isOutput=False)
    w2f_in = nc.declare_dram_parameter("w2f", [128, 2 * 54], dt.float32, isOutput=False)
    b1_in = nc.declare_dram_parameter("b1", [128, 2], dt.float32, isOutput=False)
    b2_in = nc.declare_dram_parameter("b2", [54, 1], dt.float32, isOutput=False)
    acy_in = nc.declare_dram_parameter("acy", [9, NPOS], dt.float32, isOutput=False)
    acx_in = nc.declare_dram_parameter("acx", [9, NPOS], dt.float32, isOutput=False)
    ah_in = nc.declare_dram_parameter("ah", [9, 1], dt.float32, isOutput=False)
    aw_in = nc.declare_dram_parameter("aw", [9, 1], dt.float32, isOutput=False)

    o_scores = nc.declare_dram_parameter("o_scores", [54, NPOS], dt.float32, isOutput=True)
    o_keys = nc.declare_dram_parameter("o_keys", [9, NPOS], dt.int32, isOutput=True)
    o_boxes = nc.declare_dram_parameter("o_boxes", [36, NPOS], dt.float32, isOutput=True)

    with TileContext(nc) as tc:
        with (
            tc.tile_pool(name="sb", bufs=1) as sb,
            tc.tile_pool(name="ps", bufs=2, space="PSUM") as ps,
        ):
            tx = sb.tile([128, 972], dt.float32, tag="tx")
            nc.sync.dma_start(out=tx[:], in_=x_in[:])
            tw2 = sb.tile([128, 324], dt.float16, tag="tw2")
            nc.sync.dma_start(out=tw2[:], in_=w2abc_in[:])
            # weights DMA'd round-major so stage-1 round 0 starts after 1/3 lands
            tw1a = sb.tile([128, 4608], dt.float16, tag="tw1a")
            tw1b = sb.tile([128, 4608], dt.float16, tag="tw1b")
            tw1c = sb.tile([128, 4608], dt.float16, tag="tw1c")
            for rnd in range(3):
                for t_, p_ in ((tw1a, w1a_in), (tw1b, w1b_in), (tw1c, w1c_in)):
                    for cc in range(2):
                        s0 = cc * 2304 + rnd * 768
                        nc.sync.dma_start(out=t_[:, s0:s0 + 768], in_=p_[:, s0:s0 + 768])
            tw2f = sb.tile([128, 108], dt.float32, tag="tw2f")
            nc.sync.dma_start(out=tw2f[:], in_=w2f_in[:])
            tb1 = sb.tile([128, 2], dt.float32, tag="tb1")
            nc.sync.dma_start(out=tb1[:], in_=b1_in[:])
            tb2 = sb.tile([54, 1], dt.float32, tag="tb2")
            nc.sync.dma_start(out=tb2[:], in_=b2_in[:])
            tacy = sb.tile([9, NPOS], dt.float32, tag="tacy")
            tacx = sb.tile([9, NPOS], dt.float32, tag="tacx")
            tah = sb.tile([9, 1], dt.float32, tag="tah")
            taw = sb.tile([9, 1], dt.float32, tag="taw")
            nc.sync.dma_start(out=tacy[:], in_=acy_in[:])
            nc.sync.dma_start(out=tacx[:], in_=acx_in[:])
            nc.sync.dma_start(out=tah[:], in_=ah_in[:])
            nc.sync.dma_start(out=taw[:], in_=aw_in[:])

            # ---- x fixed-point slices ----
            t8 = sb.tile([128, 972], dt.float32, tag="t8")
            nc.vector.tensor_scalar(t8[:], tx[:], float(2.0 ** 8), MAGIC, op.mult, op.add)
            nc.vector.tensor_scalar(t8[:], t8[:], MAGIC, None, op.subtract)
            xa16 = sb.tile([128, 972], dt.float16, tag="xa16")
            nc.scalar.copy(xa16[:], t8[:])
            u = sb.tile([128, 972], dt.float32, tag="u")
            v = sb.tile([128, 972], dt.float32, tag="vv")
            nc.vector.tensor_scalar(v[:], tx[:], float(2.0 ** 19), None, op.mult)
            nc.vector.scalar_tensor_tensor(u[:], t8[:], float(-(2.0 ** 11)), v[:], op.mult, op.add)
            t19 = sb.tile([128, 972], dt.float32, tag="t19")
            nc.vector.tensor_scalar(t19[:], u[:], MAGIC, MAGIC, op.add, op.subtract)
            xb16 = sb.tile([128, 972], dt.float16, tag="xb16")
            nc.scalar.copy(xb16[:], t19[:])
            nc.vector.tensor_scalar(v[:], tx[:], float(2.0 ** 30), None, op.mult)
            nc.vector.scalar_tensor_tensor(u[:], t8[:], float(-(2.0 ** 22)), v[:], op.mult, op.add)
            nc.vector.scalar_tensor_tensor(u[:], t19[:], float(-(2.0 ** 11)), u[:], op.mult, op.add)
            nc.vector.tensor_scalar(u[:], u[:], MAGIC, MAGIC, op.add, op.subtract)
            xc16 = sb.tile([128, 972], dt.float16, tag="xc16")
            nc.scalar.copy(xc16[:], u[:])


            # ---- stage 1: W_eff slices ----
            # L1: per chunk 128 cols: [Wa(0:54) | pad | Wra(64:118) | pad]
            # L1b: per chunk 9 cols: Wrb (obj)
            # L2: per chunk 80 cols: [Wa(0:54) | pad | Wra-obj(64:73) | pad]
            # L3: per chunk 9 cols: Wa-obj
            L1 = sb.tile([128, 18 * 128], dt.float16, tag="L1")
            L1b = sb.tile([128, 18 * 9], dt.float16, tag="L1b")
            L2 = sb.tile([128, 18 * 80], dt.float16, tag="L2")
            L3 = sb.tile([128, 18 * 9], dt.float16, tag="L3")
            nc.vector.memset(L1[:], 0)
            nc.vector.memset(L2[:], 0)
            L1v = L1[:].rearrange("p (c w) -> p c w", w=128)
            L2v = L2[:].rearrange("p (c w) -> p c w", w=80)
            L1bv = L1b[:].rearrange("p (c w) -> p c w", w=9)
            L3v = L3[:].rearrange("p (c w) -> p c w", w=9)
            ROUNDS = (4, 4, 4, 3, 3)
            j0 = 0
            for rnd, nch in enumerate(ROUNDS):
                pstile = ps.tile([128, 2048], dt.float32, tag="arena")
                psv = pstile[:].rearrange("p (c w) -> p c w", w=512)
                for jj in range(nch):
                    j = j0 + jj
                    base = jj * 512
                    for cc in range(2):
                        lhsA = tw1a[:, cc * 2304 + j * 128:cc * 2304 + (j + 1) * 128]
                        lhsB = tw1b[:, cc * 2304 + j * 128:cc * 2304 + (j + 1) * 128]
                        lhsC = tw1c[:, cc * 2304 + j * 128:cc * 2304 + (j + 1) * 128]
                        nc.tensor.matmul(pstile[:, base:base + 162], lhsA,
                                         tw2[:, cc * 162:cc * 162 + 162],
                                         start=(cc == 0), stop=False)
                        nc.tensor.matmul(pstile[:, base + 54:base + 162], lhsB,
                                         tw2[:, cc * 162:cc * 162 + 108],
                                         start=False, stop=False)
                        nc.tensor.matmul(pstile[:, base + 108:base + 162], lhsC,
                                         tw2[:, cc * 162:cc * 162 + 54],
                                         start=False, stop=(cc == 1))
                sl = slice(j0, j0 + nch)
                j0 += nch
                hv = psv[:, 0:nch, :]
                T0 = sb.tile([128, nch, 54], dt.float32, tag="T0")
                nc.scalar.mul(T0[:], hv[:, :, 0:54], float(2.0 ** -28))
                WaF = sb.tile([128, nch, 54], dt.float32, tag="WaF")
                nc.vector.tensor_scalar(WaF[:], T0[:], float(2.0 ** 14), MAGIC, op.mult, op.add)
                nc.vector.tensor_scalar(WaF[:], WaF[:], MAGIC, None, op.subtract)
                nc.vector.tensor_copy(L1v[:, sl, 0:54], WaF[:])
                nc.scalar.copy(L2v[:, sl, 0:54], WaF[:])
                nc.scalar.copy(L3v[:, sl, 0:9], WaF[:, :, 0:9])
                Wr1 = sb.tile([128, nch, 54], dt.float32, tag="Wr1")
                nc.vector.scalar_tensor_tensor(Wr1[:], WaF[:], float(-(2.0 ** -14)), T0[:], op.mult, op.add)
                lo = sb.tile([128, nch, 54], dt.float32, tag="lo")
                nc.scalar.mul(lo[:], hv[:, :, 54:108], float(2.0 ** -40))
                nc.vector.scalar_tensor_tensor(lo[:], hv[:, :, 108:162], float(2.0 ** -52), lo[:], op.mult, op.add)
                nc.vector.tensor_tensor(out=lo[:], in0=lo[:], in1=Wr1[:], op=op.add)
                WraF = sb.tile([128, nch, 54], dt.float32, tag="WraF")
                nc.vector.tensor_scalar(WraF[:], lo[:], float(2.0 ** 25), MAGIC, op.mult, op.add)
                nc.vector.tensor_scalar(WraF[:], WraF[:], MAGIC, None, op.subtract)
                nc.vector.tensor_copy(L1v[:, sl, 64:118], WraF[:])
                nc.scalar.copy(L2v[:, sl, 64:73], WraF[:, :, 0:9])
                Wrb = sb.tile([128, nch, 9], dt.float32, tag="Wrb")
                nc.vector.scalar_tensor_tensor(Wrb[:], WraF[:, :, 0:9], float(-(2.0 ** -25)),
                                               lo[:, :, 0:9], op.mult, op.add)
                nc.vector.tensor_scalar(Wrb[:], Wrb[:], float(2.0 ** 36), MAGIC, op.mult, op.add)
                nc.vector.tensor_scalar(Wrb[:], Wrb[:], MAGIC, None, op.subtract)
                nc.vector.tensor_copy(L1bv[:, sl, 0:9], Wrb[:])

            # ---- bias offs = w2^T b1 + b2 ----
            psOt = ps.tile([128, 2048], dt.float32, tag="arena")
            psO = psOt[0:54, 0:1]
            for cc in range(2):
                nc.tensor.matmul(psO, tw2f[:, cc * 54:(cc + 1) * 54], tb1[:, cc:cc + 1],
                                 start=(cc == 0), stop=(cc == 1))
            offs = sb.tile([54, 1], dt.float32, tag="offs")
            nc.vector.tensor_tensor(out=offs[:], in0=psO, in1=tb2[:], op=op.add)

            # ---- stage 2: 4 streams into 4 banks ----
            pstile = ps.tile([128, 2048], dt.float32, tag="arena")
            psA = pstile[0:118, 0:NPOS]          # xa x [Wa|pad|Wra]
            psAb = pstile[0:9, 512:512 + NPOS]   # xa x Wrb (obj)
            psB = pstile[0:73, 1024:1024 + NPOS]  # xb x [Wa|pad|Wra-obj]
            psC = pstile[0:9, 1536:1536 + NPOS]  # xc x Wa-obj
            xav = xa16[:].rearrange("p (c r q) -> p c r q", c=2, r=9)
            xbv = xb16[:].rearrange("p (c r q) -> p c r q", c=2, r=9)
            xcv = xc16[:].rearrange("p (c r q) -> p c r q", c=2, r=9)
            for tap in range(9):
                dy, dx = tap // 3, tap % 3
                for cc in range(2):
                    j = tap * 2 + cc
                    st = (j == 0)
                    sp = (j == 17)
                    ra = xav[:, cc, dy:dy + 7, dx:dx + 52]
                    rb = xbv[:, cc, dy:dy + 7, dx:dx + 52]
                    rc = xcv[:, cc, dy:dy + 7, dx:dx + 52]
                    nc.tensor.matmul(psA, L1[:, j * 128:j * 128 + 118], ra, start=st, stop=sp)
                    nc.tensor.matmul(psAb, L1b[:, j * 9:(j + 1) * 9], ra, start=st, stop=sp)
                    nc.tensor.matmul(psB, L2[:, j * 80:j * 80 + 73], rb, start=st, stop=sp)
                    nc.tensor.matmul(psC, L3[:, j * 9:(j + 1) * 9], rc, start=st, stop=sp)

            # ---- score combine ----
            c1 = sb.tile([54, NPOS], dt.float32, tag="c1")
            nc.scalar.mul(c1[:], pstile[64:118, 0:NPOS], float(2.0 ** -33))
            Sres = sb.tile([54, NPOS], dt.float32, tag="Sres")
            nc.vector.scalar_tensor_tensor(Sres[:], pstile[0:54, 1024:1024 + NPOS],
                                           float(2.0 ** -33), c1[:], op.mult, op.add)
            e1 = sb.tile([9, NPOS], dt.float32, tag="e1")
            nc.scalar.mul(e1[:], psAb, float(2.0 ** -44))
            nc.vector.scalar_tensor_tensor(e1[:], pstile[64:73, 1024:1024 + NPOS],
                                           float(2.0 ** -44), e1[:], op.mult, op.add)
            nc.vector.scalar_tensor_tensor(e1[:], psC, float(2.0 ** -44), e1[:], op.mult, op.add)
            nc.vector.tensor_tensor(out=Sres[0:9, :], in0=Sres[0:9, :], in1=e1[:], op=op.add)
            nc.vector.tensor_scalar(Sres[:], Sres[:], offs[:], None, op.add)
            S54 = sb.tile([54, NPOS], dt.float32, tag="S54")
            nc.vector.scalar_tensor_tensor(S54[:], pstile[0:54, 0:NPOS], float(2.0 ** -22),
                                           Sres[:], op.mult, op.add)
            nc.sync.dma_start(out=o_scores[:], in_=S54[:])

            # ---- int32 keys (before masking) ----
            kr = sb.tile([9, NPOS], dt.float32, tag="kr")
            nc.vector.tensor_scalar(kr[:], Sres[0:9, :], float(2.0 ** 31), MAGIC, op.mult, op.add)
            nc.vector.tensor_scalar(kr[:], kr[:], MAGIC, None, op.subtract)
            kri = sb.tile([9, NPOS], dt.int32, tag="kri")
            nc.vector.tensor_copy(kri[:], kr[:])
            k1 = sb.tile([9, NPOS], dt.int32, tag="k1")
            nc.vector.tensor_copy(k1[:], pstile[0:9, 0:NPOS])
            key = sb.tile([9, NPOS], dt.int32, tag="key")
            nc.vector.tensor_scalar(key[:], k1[:], 512, None, op.mult)
            nc.vector.tensor_tensor(out=key[:], in0=key[:], in1=kri[:], op=op.add)

            # ---- decode boxes ----
            Sl = []
            for d in range(4):
                t = sb.tile([9, NPOS], dt.float32, tag=f"Sl{d}")
                nc.sync.dma_start(out=t[:], in_=S54[18 + d * 9:27 + d * 9, :])
                Sl.append(t)
            cy = sb.tile([9, NPOS], dt.float32, tag="cy")
            cx = sb.tile([9, NPOS], dt.float32, tag="cx")
            nc.vector.scalar_tensor_tensor(cy[:], Sl[0][:], tah[:], tacy[:], op.mult, op.add)
            nc.vector.scalar_tensor_tensor(cx[:], Sl[1][:], taw[:], tacx[:], op.mult, op.add)
            eh = sb.tile([9, NPOS], dt.float32, tag="eh")
            ew = sb.tile([9, NPOS], dt.float32, tag="ew")
            for (srct, dstt) in ((Sl[2], eh), (Sl[3], ew)):
                nc.vector.tensor_scalar(dstt[:], srct[:], float(EXP_CO[0]), float(EXP_CO[1]),
                                        op.mult, op.add)
                for ci in range(2, 6):
                    nc.vector.tensor_tensor(out=dstt[:], in0=dstt[:], in1=srct[:], op=op.mult)
                    nc.vector.tensor_scalar(dstt[:], dstt[:], float(EXP_CO[ci]), None, op.add)
            nc.vector.tensor_scalar(eh[:], eh[:], tah[:], 0.5, op.mult, op.mult)
            nc.vector.tensor_scalar(ew[:], ew[:], taw[:], 0.5, op.mult, op.mult)
            bco = []
            for (ctr, half, sgn) in ((cy, eh, op.subtract), (cx, ew, op.subtract),
                                     (cy, eh, op.add), (cx, ew, op.add)):
                t = sb.tile([9, NPOS], dt.float32, tag=f"bco{len(bco)}")
                nc.vector.tensor_tensor(out=t[:], in0=ctr[:], in1=half[:], op=sgn)
                nc.vector.tensor_scalar(t[:], t[:], 0.0, float(IMG_W), op.max, op.min)
                nc.sync.dma_start(out=o_boxes[len(bco) * 9:(len(bco) + 1) * 9, :], in_=t[:])
                bco.append(t)
            hs = sb.tile([9, NPOS], dt.float32, tag="hs")
            ws = sb.tile([9, NPOS], dt.float32, tag="ws")
            nc.vector.tensor_tensor(out=hs[:], in0=bco[2][:], in1=bco[0][:], op=op.subtract)
            nc.vector.tensor_tensor(out=ws[:], in0=bco[3][:], in1=bco[1][:], op=op.subtract)
            nc.vector.tensor_scalar(hs[:], hs[:], 16.0, None, op.is_ge)
            nc.vector.tensor_scalar(ws[:], ws[:], 16.0, None, op.is_ge)
            nc.vector.tensor_tensor(out=hs[:], in0=hs[:], in1=ws[:], op=op.mult)
            vi = sb.tile([9, NPOS], dt.int32, tag="vi")
            nc.vector.tensor_copy(vi[:], hs[:])
            nc.vector.tensor_scalar(key[:], key[:], 2 ** 30, None, op.add)
            nc.vector.tensor_tensor(out=key[:], in0=key[:], in1=vi[:], op=op.mult)
            nc.vector.tensor_scalar(key[:], key[:], 2 ** 30, None, op.subtract)
            nc.sync.dma_start(out=o_keys[:], in_=key[:])

    nc.compile()
    _BUILD_CACHE["nc"] = nc
    return nc


def _host_inputs(x, conv1_w, conv1_b, reg_w, reg_b, cls_w, cls_b):
    x = np.asarray(x, f32)[0]
    w1 = np.asarray(conv1_w, f32)
    b1 = np.asarray(conv1_b, f32)
    wr = np.asarray(reg_w, f32)[:, :, 0, 0]
    br = np.asarray(reg_b, f32)
    wc = np.asarray(cls_w, f32)[:, :, 0, 0]
    bc = np.asarray(cls_b, f32)

    w2_all = np.concatenate([wr, wc], 0)
    b2_all = np.concatenate([br, bc], 0)
    w2p = np.ascontiguousarray(w2_all[CHAN_PERM].T.astype(f32))
    b2p = b2_all[CHAN_PERM].astype(f32)

    w1p = np.ascontiguousarray(w1.transpose(0, 2, 3, 1).reshape(256, 2304).astype(f32))
    w1a, w1b, w1c = _slice3(w1p, 14, 26, 38)
    w2a, w2b, w2c = _slice3(w2p, 14, 26, 38)

    def two_chunk(m):
        F = m.shape[1]
        out = np.empty((128, 2 * F), m.dtype)
        out[:, :F] = m[:128]
        out[:, F:] = m[128:]
        return out

    w1a_h = two_chunk(w1a).astype(np.float16)
    w1b_h = two_chunk(w1b).astype(np.float16)
    w1c_h = two_chunk(w1c).astype(np.float16)
    w2abc_h = two_chunk(np.concatenate([w2a, w2b, w2c], 1)).astype(np.float16)
    w2f_h = two_chunk(w2p).astype(f32)
    b1_h = np.ascontiguousarray(b1.reshape(2, 128).T.astype(f32))
    b2_h = b2p.reshape(54, 1)

    vals, ahv, awv = _make_anchor_geom()
    ah_h = ahv.reshape(9, 1).astype(f32)
    aw_h = awv.reshape(9, 1).astype(f32)

    xpad = np.zeros((256, 58, 54), f32)
    xpad[:, 1:53, 1:53] = x
    valsf = vals.astype(f32)
    in_maps = []
    for c in range(N_CORES):
        r0 = c * R
        stripe = xpad[:, r0:r0 + 9, :]
        x_h = two_chunk(np.ascontiguousarray(stripe.reshape(256, 9 * 54)))
        acy = np.zeros((9, NPOS), f32)
        acx = np.zeros((9, NPOS), f32)
        for rr in range(R):
            gr = r0 + rr
            acy[:, rr * 52:(rr + 1) * 52] = valsf[gr] if gr < FEAT_W else f32(0)
            acx[:, rr * 52:(rr + 1) * 52] = valsf[None, :FEAT_W]
        in_maps.append({
            "x_in": x_h, "w1a": w1a_h, "w1b": w1b_h, "w1c": w1c_h,
            "w2abc": w2abc_h, "w2f": w2f_h, "b1": b1_h, "b2": b2_h,
            "acy": acy, "acx": acx, "ah": ah_h, "aw": aw_h,
        })
    return in_maps


def _host_nms(keys, boxes):
    order = np.argsort(-keys.astype(np.int64), kind="stable")[:PRE_NMS]
    b = boxes[order]
    areas = (b[:, 3] - b[:, 1] + f32(1)) * (b[:, 2] - b[:, 0] + f32(1))
    N = len(order)
    idx = np.arange(N)
    supp = np.zeros(N, bool)
    kept = []
    for i in range(N):
        if supp[i]:
            continue
        kept.append(i)
        if len(kept) == POST_NMS:
            break
        bi = b[i]
        yy1 = np.maximum(bi[0], b[:, 0])
        xx1 = np.maximum(bi[1], b[:, 1])
        yy2 = np.minimum(bi[2], b[:, 2])
        xx2 = np.minimum(bi[3], b[:, 3])
        inter = np.maximum(f32(0), xx2 - xx1 + f32(1)) * np.maximum(f32(0), yy2 - yy1 + f32(1))
        iou = inter / (areas[i] + areas - inter)
        supp |= (iou > f32(0.7)) & (idx > i)
    out = np.zeros((POST_NMS, 4), f32)
    k = np.array(kept[:POST_NMS])
    out[:len(k)] = b[k]
    return out


TRACE = False
LAST_RESULT = [None]


def kernel(x, conv1_w, conv1_b, reg_w, reg_b, cls_w, cls_b):
    from concourse.bass_utils import run_bass_kernel_spmd
    nc = _build()
    in_maps = _host_inputs(x, conv1_w, conv1_b, reg_w, reg_b, cls_w, cls_b)
    res = run_bass_kernel_spmd(nc, in_maps, list(range(N_CORES)), trace=TRACE)
    LAST_RESULT[0] = res

    scores = np.zeros((54, 56 * 52), f32)
    keys = np.zeros((9, 56 * 52), np.int32)
    boxes = np.zeros((36, 56 * 52), f32)
    for c in range(N_CORES):
        r = res.results[c]
        scores[:, c * NPOS:(c + 1) * NPOS] = r["o_scores"]
        keys[:, c * NPOS:(c + 1) * NPOS] = np.asarray(r["o_keys"]).view(np.int32)
        boxes[:, c * NPOS:(c + 1) * NPOS] = r["o_boxes"]
    npos = FEAT_W * FEAT_W
    scores = scores[:, :npos]
    keys = keys[:, :npos]
    boxes = boxes[:, :npos]

    locs = np.empty((npos, NA, 4), f32)
    for d in range(4):
        locs[:, :, d] = scores[18 + d * 9:27 + d * 9, :].T
    locs = locs.reshape(1, npos * NA, 4)
    cls_scores = np.empty((npos, NA, 2), f32)
    cls_scores[:, :, 0] = scores[9:18, :].T
    cls_scores[:, :, 1] = scores[0:9, :].T
    cls_scores = cls_scores.reshape(1, npos * NA, 2)
    obj = np.ascontiguousarray(cls_scores[:, :, 1].reshape(1, npos * NA))

    kflat = keys.T.reshape(-1)
    bflat = np.stack([boxes[0:9].T.reshape(-1), boxes[9:18].T.reshape(-1),
                      boxes[18:27].T.reshape(-1), boxes[27:36].T.reshape(-1)], 1)
    rois = _host_nms(kflat, bflat)
    return (rois, locs, cls_scores, obj, cls_scores)
